# revision 16
# baseline (speedup 1.0000x reference)
"""Trainium2 Bass kernel for nn_CorrLayer (Autoformer AutoCorrelation layer).

Contract: kernel(**inputs) takes FULL inputs (queries/keys/values [4,2048,1024],
Wq/bq/Wk/bk/Wv/bv/Wo/bo) and returns the FULL output [4,2048,1024], running the
compute on 8 NeuronCores.

Sharding: core = 2*b + half.  Each core-pair handles one batch b:
  - q/k projections + DFT products are split by channel half (c-split);
    the per-frequency channel-sums R,S are all-reduced pairwise (8.7 KB).
  - the output projection + time-delay gather are split by output-column half.

Device algorithm (per core), matmul operands mostly bf16 (PSUM accum fp32):
  1. Host folds inputs by the DFT even/odd symmetry: for l'=1..1023,
     x+[l'] = x[l'] + x[2048-l'], x-[l'] = x[l'] - x[2048-l']; x+[1024] =
     x[1024]; plus the l=0 column.  This halves the DFT contraction length.
  2. q+/q-/k+/k- = projections of folded inputs ([l',c] tiles); q0/k0 rows.
  3. DFT-as-matmul on folded data, 8 f-chunks of 128 (f=0..1023):
     A = cosF^T q+ (+ q0 broadcast, + L*bq at f=0 only), As = sinF^T q-;
     R[f] = sum_c A*C, S[f] = sum_c As*Cs.  Nyquist bin f=1024 via the
     alternating-sign column (exact).
  4. Pairwise AllReduce of (R,S) [128,17].
  5. Wfused = Wv @ Wo[:,half] on device (bf16), so the value path needs no
     separate v projection: VPT[j,l] = Wfused^T xv^T directly (the duplicated
     full-channel v-projection and its DRAM spill are gone).
  6. mean corr mv[l] via factored irfft as one [4,512] PSUM matmul chain.
  7. top-16 of mv via two max8 rounds; softmax over top-15.
  8. out^T[j,l] = sum_k w_k VPT2[j, l+delta_k] via PSUM-accumulated
     scaled-identity matmuls with register-offset dynamic slices; + bo + bv@Wo.
Host: input transposes + folds, DFT constant matrices, output assembly.
"""
import math
import numpy as np
import ml_dtypes

import concourse.bass as bass
import concourse.bacc as bacc
import concourse.mybir as mybir
import concourse.tile as tile
from concourse.bass_utils import run_bass_kernel_spmd

F32 = mybir.dt.float32
F32R = mybir.dt.float32r
BF16 = mybir.dt.bfloat16
U32 = mybir.dt.uint32
AF = mybir.ActivationFunctionType
NPBF16 = ml_dtypes.bfloat16

B, L, D = 4, 2048, 1024
H, DK = 16, 64
CH = 512            # channels per core (c-split half)
NFT = 8             # f chunks of 128 -> bins 0..1023; Nyquist 1024 separate
NLT = 8             # l' tiles (l' = 1..1024 folded)
NDT = D // 128      # 8 d-tiles
TOPK = 15           # int(2*log(2048))
NTAP = 15
N_CORES = 8

_cache = {}


def _host_constants():
    f = np.arange(1024)
    lp = np.arange(1, 1025)                  # l' = 1..1024, j = l'-1
    ang = 2.0 * np.pi * np.outer(lp, f) / L
    cosF = np.cos(ang)                       # [1024 j, 1024 f]
    sinF = np.sin(ang)
    # SBUF chunk layout [ft, p, lt*128+fc] with p = j%128, lt = j//128
    def chunkify(m):
        return np.ascontiguousarray(
            m.reshape(NLT, 128, NFT, 128).transpose(2, 1, 0, 3)
            .reshape(NFT, 128, NLT * 128).astype(NPBF16))
    cosF8 = chunkify(cosF)
    sinF8 = chunkify(sinF)
    altcol = np.ascontiguousarray(
        ((-1.0) ** lp).reshape(NLT, 128).T.astype(NPBF16))   # [128, 8]

    # irfft, factored over l = a*512 + b:
    #   mv[a*512+b] = sum_f U[f,a] cosB[f,b] + V[f,a] sinB[f,b]
    #   U = wf(R cosA + S sinA), V = wf(S cosA - R sinA)
    wf = np.full(1025, 2.0 / L, np.float64)
    wf[0] = 1.0 / L
    wf[1024] = 1.0 / L
    wf = wf / (H * DK)   # fold the channel-mean into the inverse transform
    a4 = np.arange(4)
    b512 = np.arange(512)
    cosA = np.cos(np.pi * np.outer(f, a4) / 2.0)
    sinA = np.sin(np.pi * np.outer(f, a4) / 2.0)
    # [128 p, 32] with col = ft*4 + a
    wca = (wf[:1024, None] * cosA).astype(np.float32)
    wsa = (wf[:1024, None] * sinA).astype(np.float32)
    wcaP = np.ascontiguousarray(
        wca.reshape(NFT, 128, 4).transpose(1, 0, 2).reshape(128, NFT * 4))
    wsaP = np.ascontiguousarray(
        wsa.reshape(NFT, 128, 4).transpose(1, 0, 2).reshape(128, NFT * 4))
    cosB3 = np.ascontiguousarray(
        np.cos(2.0 * np.pi * np.outer(f, b512) / L)
        .astype(np.float32).reshape(NFT, 128, 512))
    sinB3 = np.ascontiguousarray(
        np.sin(2.0 * np.pi * np.outer(f, b512) / L)
        .astype(np.float32).reshape(NFT, 128, 512))
    altb_row = np.ascontiguousarray(
        (wf[1024] * ((-1.0) ** b512)).astype(np.float32)[None, :])  # [1,512]
    return cosF8, sinF8, altcol, wcaP, wsaP, cosB3, sinB3, altb_row


def _build_program():
    nc = bacc.Bacc("TRN2", target_bir_lowering=False, debug=False,
                   enable_asserts=False, num_devices=N_CORES)

    def din(name, shape, dt):
        return nc.dram_tensor(name, shape, dt, kind="ExternalInput").ap()

    v = {}
    for nm in ("xqp", "xqm", "xkp", "xkm"):
        v[nm] = din(nm, [D, 1024], BF16)
    v["xq0"] = din("xq0", [128, NDT], BF16)
    v["xk0"] = din("xk0", [128, NDT], BF16)
    v["xtv"] = din("xtv", [D, L], BF16)
    v["wq"] = din("wq", [D, CH], BF16)
    v["wk"] = din("wk", [D, CH], BF16)
    v["wvt"] = din("wvt", [D, D], BF16)
    v["wo"] = din("wo", [D, CH], BF16)
    v["bv_dt"] = din("bv_dt", [128, NDT], BF16)
    v["bqL_row"] = din("bqL_row", [1, CH], F32)
    v["bkL_row"] = din("bkL_row", [1, CH], F32)
    v["bo_cols"] = din("bo_cols", [128, 4], F32)
    v["cosF8"] = din("cosF8", [NFT, 128, NLT * 128], BF16)
    v["sinF8"] = din("sinF8", [NFT, 128, NLT * 128], BF16)
    v["altcol"] = din("altcol", [128, NLT], BF16)
    v["cosB3"] = din("cosB3", [NFT, 128, 512], F32R)
    v["sinB3"] = din("sinB3", [NFT, 128, 512], F32R)
    v["wcaP"] = din("wcaP", [128, 4 * NFT], F32)
    v["wsaP"] = din("wsaP", [128, 4 * NFT], F32)
    v["altb_row"] = din("altb_row", [1, 512], F32R)
    v["ones_row"] = din("ones_row", [1, 128], F32R)
    v["ident"] = din("ident", [128, 128], BF16)
    v["out_t"] = nc.dram_tensor("out_t", [CH, L], BF16,
                                kind="ExternalOutput").ap()

    with tile.TileContext(nc) as tc:
        with tc.tile_pool(name="dram", bufs=1, space="DRAM") as dram_pool:
            v["rs_in"] = dram_pool.tile([128, 2 * NFT + 1], F32,
                                        name="rs_in")
            v["rs_out"] = dram_pool.tile([128, 2 * NFT + 1], F32,
                                         name="rs_out")
            v["delta_dram"] = dram_pool.tile([1, CH], F32, name="delta_dram")
            v["mv_dram"] = dram_pool.tile([1, L], F32, name="mv_dram")
            _build_body(nc, tc, v)
    nc.compile()
    return nc


def _build_body(nc, tc, v):
    from contextlib import ExitStack
    stack = ExitStack()

    const_pool = stack.enter_context(tc.tile_pool(name="const", bufs=1))
    xq0_sb = const_pool.tile([128, NDT], BF16, tag="x0", bufs=2)
    nc.scalar.dma_start(xq0_sb[:], v["xq0"])
    xk0_sb = const_pool.tile([128, NDT], BF16, tag="x0", bufs=2)
    nc.scalar.dma_start(xk0_sb[:], v["xk0"])
    ones_sb = const_pool.tile([1, 128], F32R, tag="ones")
    nc.scalar.dma_start(ones_sb[:], v["ones_row"])
    ident_sb = const_pool.tile([128, 128], BF16, tag="ident")
    nc.scalar.dma_start(ident_sb[:], v["ident"])
    bo_sb = const_pool.tile([128, 4], F32, tag="bo")
    nc.scalar.dma_start(bo_sb[:], v["bo_cols"])
    bqL_sb = const_pool.tile([1, CH], F32, tag="bql")
    nc.scalar.dma_start(bqL_sb[:], v["bqL_row"])
    bkL_sb = const_pool.tile([1, CH], F32, tag="bkl")
    nc.scalar.dma_start(bkL_sb[:], v["bkL_row"])
    altcol_sb = const_pool.tile([128, NLT], BF16, tag="altc")
    nc.scalar.dma_start(altcol_sb[:], v["altcol"])
    bv_sb = const_pool.tile([128, NDT], BF16, tag="bv")
    nc.scalar.dma_start(bv_sb[:], v["bv_dt"])
    rs_sb = const_pool.tile([128, 2 * NFT + 1], F32, tag="rs")
    rs2_sb = const_pool.tile([128, 2 * NFT + 1], F32, tag="rs2")
    q0row = const_pool.tile([1, CH], F32R, tag="q0r", bufs=2)
    k0row = const_pool.tile([1, CH], F32R, tag="q0r", bufs=2)
    q0bc = const_pool.tile([128, CH], F32, tag="q0bc", bufs=3)
    k0bc = const_pool.tile([128, CH], F32, tag="q0bc", bufs=3)
    q0bc0 = const_pool.tile([128, CH], F32, tag="q0bc", bufs=3)
    wI = const_pool.tile([128, NTAP * 128], BF16, tag="wI")
    bo2_sb = const_pool.tile([128, 4], F32, tag="bo2")
    inv_sb = const_pool.tile([128, 1], F32, tag="inv")
    wca_sb = const_pool.tile([128, 4 * NFT], F32, tag="wca")
    nc.scalar.dma_start(wca_sb[:], v["wcaP"])
    wsa_sb = const_pool.tile([128, 4 * NFT], F32, tag="wsa")
    nc.scalar.dma_start(wsa_sb[:], v["wsaP"])
    altb_sb = const_pool.tile([1, 512], F32R, tag="altb")
    nc.scalar.dma_start(altb_sb[:], v["altb_row"])

    # =============== Stage A: folded projections ===============
    qp_sb = [None] * NLT
    qm_sb = [None] * NLT
    kp_sb = [None] * NLT
    km_sb = [None] * NLT
    qk_pool = stack.enter_context(tc.tile_pool(name="qk", bufs=4 * NLT))
    with tc.tile_pool(name="xin", bufs=9) as xin_pool, \
         tc.tile_pool(name="wqk", bufs=1) as w_pool, \
         tc.tile_pool(name="psumA", bufs=8, space="PSUM") as psum_a:
        wq_t, wk_t = [], []
        qp_x = []
        for dt_i in range(NDT):
            wt = w_pool.tile([128, CH], BF16, tag="wq", bufs=NDT,
                             name=f"wq{dt_i}")
            nc.sync.dma_start(wt[:], v["wq"][dt_i * 128:(dt_i + 1) * 128, :])
            wq_t.append(wt)
            xt_ = xin_pool.tile([128, 1024], BF16, tag="xin",
                                name=f"qpx{dt_i}")
            eng = nc.sync if dt_i % 2 == 0 else nc.scalar
            eng.dma_start(xt_[:], v["xqp"][dt_i * 128:(dt_i + 1) * 128, :])
            qp_x.append(xt_)
            wt = w_pool.tile([128, CH], BF16, tag="wk", bufs=NDT,
                             name=f"wk{dt_i}")
            nc.scalar.dma_start(wt[:], v["wk"][dt_i * 128:(dt_i + 1) * 128, :])
            wk_t.append(wt)
        for name, src, w_t, dst in (("qp", "xqp", wq_t, qp_sb),
                                    ("qm", "xqm", wq_t, qm_sb),
                                    ("kp", "xkp", wk_t, kp_sb),
                                    ("km", "xkm", wk_t, km_sb)):
            if name == "qp":
                x_t = qp_x
            else:
                x_t = []
                for dt_i in range(NDT):
                    xt_ = xin_pool.tile([128, 1024], BF16, tag="xin",
                                        name=f"{name}x{dt_i}")
                    eng = nc.sync if dt_i % 2 == 0 else nc.scalar
                    eng.dma_start(xt_[:],
                                  v[src][dt_i * 128:(dt_i + 1) * 128, :])
                    x_t.append(xt_)
            for ltg in range(2):
                ps = [psum_a.tile([128, CH], F32, tag="ps", bufs=6,
                                  name=f"p{name}{ltg}_{i}") for i in range(4)]
                for dt_i in range(NDT):
                    for li in range(4):
                        lt = ltg * 4 + li
                        nc.tensor.matmul(
                            ps[li][:],
                            x_t[dt_i][:, lt * 128:(lt + 1) * 128],
                            w_t[dt_i][:], start=(dt_i == 0),
                            stop=(dt_i == NDT - 1))
                for li in range(4):
                    t = qk_pool.tile([128, CH], BF16, tag="qk",
                                     name=f"{name}{ltg * 4 + li}")
                    nc.scalar.copy(t[:], ps[li][:])
                    dst[ltg * 4 + li] = t

        # q0/k0 rows (raw l=0 projections, no bias) + broadcast tiles
        ps_q0 = psum_a.tile([1, CH], F32, tag="ps0", bufs=2)
        ps_k0 = psum_a.tile([1, CH], F32, tag="ps0", bufs=2)
        for dt_i in range(NDT):
            nc.tensor.matmul(ps_q0[:], xq0_sb[:, dt_i:dt_i + 1], wq_t[dt_i][:],
                             start=(dt_i == 0), stop=(dt_i == NDT - 1))
        for dt_i in range(NDT):
            nc.tensor.matmul(ps_k0[:], xk0_sb[:, dt_i:dt_i + 1], wk_t[dt_i][:],
                             start=(dt_i == 0), stop=(dt_i == NDT - 1))
        nc.scalar.copy(q0row[:], ps_q0[:])
        nc.scalar.copy(k0row[:], ps_k0[:])
        ps_qb = psum_a.tile([128, CH], F32, tag="ps", bufs=6, name="psqb")
        ps_kb = psum_a.tile([128, CH], F32, tag="ps", bufs=6, name="pskb")
        nc.tensor.matmul(ps_qb[:], ones_sb[:], q0row[:], start=True, stop=True)
        nc.tensor.matmul(ps_kb[:], ones_sb[:], k0row[:], start=True, stop=True)
        nc.scalar.copy(q0bc[:], ps_qb[:])
        nc.scalar.copy(k0bc[:], ps_kb[:])
        # chunk-0 correction tile: q0bc + L*bq on the f=0 partition row
        nc.vector.tensor_copy(q0bc0[:], q0bc[:])
        nc.vector.tensor_add(q0bc0[0:1, :], q0bc[0:1, :], bqL_sb[:])

    # =============== Stage B: folded DFT + channel-summed products =========
    # trig chunks 0/1 prefetch first, then the value-path weights/inputs so
    # they sit ahead of the slot-blocked later trig chunks in the DMA queues
    trig_pool = stack.enter_context(tc.tile_pool(name="trigB", bufs=6))
    trig_tiles = {}
    for ft in range(2):
        cos_sb = trig_pool.tile([128, NLT * 128], BF16, tag="trig",
                                name=f"cos{ft}")
        nc.sync.dma_start(cos_sb[:], v["cosF8"][ft])
        sin_sb = trig_pool.tile([128, NLT * 128], BF16, tag="trig",
                                name=f"sin{ft}")
        nc.sync.dma_start(sin_sb[:], v["sinF8"][ft])
        trig_tiles[ft] = (cos_sb, sin_sb)
    wf_pool = stack.enter_context(tc.tile_pool(name="wf", bufs=1))
    xtv_pool = stack.enter_context(tc.tile_pool(name="xtv", bufs=1))
    wvt_t, wo_t, xtv_t, wf_t = [], [], [], []
    for ct in range(NDT):
        t = wf_pool.tile([128, D], BF16, tag="wvt", bufs=NDT, name=f"wvt{ct}")
        nc.sync.dma_start(t[:], v["wvt"][ct * 128:(ct + 1) * 128, :])
        wvt_t.append(t)
        t = wf_pool.tile([128, CH], BF16, tag="wo", bufs=NDT, name=f"wo{ct}")
        nc.scalar.dma_start(t[:], v["wo"][ct * 128:(ct + 1) * 128, :])
        wo_t.append(t)
        t = xtv_pool.tile([128, L], BF16, tag="xtv", bufs=NDT, name=f"xtv{ct}")
        eng = nc.sync if ct % 2 == 0 else nc.scalar
        eng.dma_start(t[:], v["xtv"][ct * 128:(ct + 1) * 128, :])
        xtv_t.append(t)

    with tc.tile_pool(name="ev", bufs=1) as ev_pool, \
         tc.tile_pool(name="psumB", bufs=8, space="PSUM") as psum_b:
        # Nyquist bin f=1024 (exact) first: its R lands in rs_sb early so the
        # collective fire is gated only by the last f-chunk
        pN1 = psum_b.tile([1, CH], F32, tag="psn", bufs=2, name="pN1")
        pN2 = psum_b.tile([1, CH], F32, tag="psn", bufs=2, name="pN2")
        for lt in range(NLT):
            st, sp = (lt == 0), (lt == NLT - 1)
            nc.tensor.matmul(pN1[:], altcol_sb[:, lt:lt + 1], qp_sb[lt][:],
                             start=st, stop=sp)
        for lt in range(NLT):
            st, sp = (lt == 0), (lt == NLT - 1)
            nc.tensor.matmul(pN2[:], altcol_sb[:, lt:lt + 1], kp_sb[lt][:],
                             start=st, stop=sp)
        eN1 = ev_pool.tile([1, CH], F32, tag="en", bufs=4)
        eN2 = ev_pool.tile([1, CH], F32, tag="en", bufs=4)
        nc.scalar.copy(eN1[:], pN1[:])
        nc.scalar.copy(eN2[:], pN2[:])
        eN1b = ev_pool.tile([1, CH], F32, tag="en", bufs=4)
        eN2b = ev_pool.tile([1, CH], F32, tag="en", bufs=4)
        nc.vector.tensor_add(eN1b[:], eN1[:], q0row[:])
        nc.vector.tensor_add(eN2b[:], eN2[:], k0row[:])
        prodN = ev_pool.tile([1, CH], F32, tag="pn")
        nc.vector.tensor_mul(prodN[:], eN1b[:], eN2b[:])
        nc.vector.memset(rs_sb[:, 2 * NFT:2 * NFT + 1], 0.0)
        nc.vector.reduce_sum(rs_sb[0:1, 2 * NFT:2 * NFT + 1], prodN[:],
                             axis=mybir.AxisListType.X)

        for ft in range(NFT):
            if ft < 2:
                cos_sb, sin_sb = trig_tiles[ft]
            else:
                cos_sb = trig_pool.tile([128, NLT * 128], BF16, tag="trig",
                                        name=f"cos{ft}")
                nc.sync.dma_start(cos_sb[:], v["cosF8"][ft])
                sin_sb = trig_pool.tile([128, NLT * 128], BF16, tag="trig",
                                        name=f"sin{ft}")
                nc.sync.dma_start(sin_sb[:], v["sinF8"][ft])

            pA = psum_b.tile([128, CH], F32, tag="ps", bufs=6, name="pA")
            pC = psum_b.tile([128, CH], F32, tag="ps", bufs=6, name="pC")
            pAs = psum_b.tile([128, CH], F32, tag="ps", bufs=6, name="pAs")
            pCs = psum_b.tile([128, CH], F32, tag="ps", bufs=6, name="pCs")
            for lt in range(NLT):
                st, sp = (lt == 0), (lt == NLT - 1)
                cs = cos_sb[:, lt * 128:(lt + 1) * 128]
                ss = sin_sb[:, lt * 128:(lt + 1) * 128]
                nc.tensor.matmul(pA[:], cs, qp_sb[lt][:], start=st, stop=sp)
                nc.tensor.matmul(pC[:], cs, kp_sb[lt][:], start=st, stop=sp)
                nc.tensor.matmul(pAs[:], ss, qm_sb[lt][:], start=st, stop=sp)
                nc.tensor.matmul(pCs[:], ss, km_sb[lt][:], start=st, stop=sp)

            eA = ev_pool.tile([128, CH], F32, tag="ev", bufs=6)
            eC = ev_pool.tile([128, CH], F32, tag="ev", bufs=6)
            eAs = ev_pool.tile([128, CH], F32, tag="ev", bufs=6)
            eCs = ev_pool.tile([128, CH], F32, tag="ev", bufs=6)
            nc.scalar.copy(eAs[:], pAs[:])
            nc.scalar.copy(eCs[:], pCs[:])
            nc.scalar.copy(eA[:], pA[:])
            nc.scalar.copy(eC[:], pC[:])
            # S-path on gpsimd (SBUF-only) in parallel with the R-path on DVE
            prodR = ev_pool.tile([128, CH], F32, tag="prod", bufs=2)
            prodS = ev_pool.tile([128, CH], F32, tag="prod", bufs=2)
            nc.gpsimd.tensor_mul(prodS[:], eAs[:], eCs[:])
            nc.vector.reduce_sum(rs_sb[:, NFT + ft:NFT + ft + 1], prodS[:],
                                 axis=mybir.AxisListType.X)
            eA2 = ev_pool.tile([128, CH], F32, tag="ev2", bufs=4)
            eC2 = ev_pool.tile([128, CH], F32, tag="ev2", bufs=4)
            nc.vector.tensor_add(eA2[:], eA[:], (q0bc0 if ft == 0 else q0bc)[:])
            nc.vector.tensor_add(eC2[:], eC[:], k0bc[:])
            nc.vector.tensor_mul(prodR[:], eA2[:], eC2[:])
            nc.vector.reduce_sum(rs_sb[:, ft:ft + 1], prodR[:],
                                 axis=mybir.AxisListType.X)

    # =============== Stage C: pairwise all-reduce of (R,S) ===============
    nc.gpsimd.dma_start(v["rs_in"][:], rs_sb[:])
    nc.gpsimd.collective_compute(
        "AllReduce", mybir.AluOpType.add,
        replica_groups=[[0, 1], [2, 3], [4, 5], [6, 7]],
        ins=[v["rs_in"].opt()], outs=[v["rs_out"].opt()])
    nc.gpsimd.dma_start(rs2_sb[:], v["rs_out"][:])

    psum_def = stack.enter_context(
        tc.tile_pool(name="psumDEF", bufs=8, space="PSUM"))
    vpt_pool = stack.enter_context(tc.tile_pool(name="vpt", bufs=1))
    vpt2 = [vpt_pool.tile([128, 2 * L], BF16, tag="vpt2", bufs=4,
                          name=f"vpt2_{i}") for i in range(4)]

    # =============== Stage W: fused Wv@Wo + value transform ===============
    # (issued after the collective fire so the PE chews it during the
    #  allreduce + reload window)
    for dt_i in range(NDT):
        ps = psum_def.tile([128, CH], F32, tag="ps", bufs=6, name=f"pwf{dt_i}")
        for ct in range(NDT):
            nc.tensor.matmul(ps[:], wvt_t[ct][:, dt_i * 128:(dt_i + 1) * 128],
                             wo_t[ct][:], start=(ct == 0), stop=(ct == NDT - 1))
        t = wf_pool.tile([128, CH], BF16, tag="wf", bufs=NDT, name=f"wf{dt_i}")
        nc.scalar.copy(t[:], ps[:])
        wf_t.append(t)
    # delta row = bv @ Wo_half -> [1,512] -> DRAM -> [128,4] -> bo2
    ps_d = psum_def.tile([1, CH], F32, tag="small", bufs=2, name="ps_d")
    for ct in range(NDT):
        nc.tensor.matmul(ps_d[:], bv_sb[:, ct:ct + 1], wo_t[ct][:],
                         start=(ct == 0), stop=(ct == NDT - 1))
    drow = const_pool.tile([1, CH], F32, tag="drow")
    nc.scalar.copy(drow[:], ps_d[:])
    nc.scalar.dma_start(v["delta_dram"][:], drow[:])
    dcols = const_pool.tile([128, 4], F32, tag="dcols")
    nc.scalar.dma_start(
        dcols[:], v["delta_dram"].rearrange("o (a p) -> p (o a)", p=128))
    nc.vector.tensor_add(bo2_sb[:], bo_sb[:], dcols[:])

    def vpt_jtile(jt, on_vector=False):
        for lch in range(4):
            ps = psum_def.tile([128, 512], F32, tag="ps", bufs=6, name=f"pv{jt}_{lch}")
            for dt_i in range(NDT):
                nc.tensor.matmul(
                    ps[:], wf_t[dt_i][:, jt * 128:(jt + 1) * 128],
                    xtv_t[dt_i][:, lch * 512:(lch + 1) * 512],
                    start=(dt_i == 0), stop=(dt_i == NDT - 1))
            if on_vector:
                nc.vector.tensor_copy(vpt2[jt][:, lch * 512:(lch + 1) * 512],
                                      ps[:])
            else:
                nc.scalar.copy(vpt2[jt][:, lch * 512:(lch + 1) * 512], ps[:])
            eng = nc.sync if lch % 2 == 0 else nc.scalar
            eng.dma_start(vpt2[jt][:, L + lch * 512:L + (lch + 1) * 512],
                          vpt2[jt][:, lch * 512:(lch + 1) * 512])

    vpt_jtile(0)
    vpt_jtile(1)

    # =============== Stage D: irfft (factored) + top-16 + weights ==========
    with tc.tile_pool(name="trigD", bufs=4) as trigd_pool, \
         tc.tile_pool(name="top", bufs=1) as top_pool:
        rrep = top_pool.tile([128, 4 * NFT], F32, tag="rrep")
        srep = top_pool.tile([128, 4 * NFT], F32, tag="srep")
        for ft in range(NFT):
            nc.vector.tensor_copy(
                rrep[:, ft * 4:(ft + 1) * 4],
                rs2_sb[:, ft:ft + 1].to_broadcast((128, 4)))
            nc.vector.tensor_copy(
                srep[:, ft * 4:(ft + 1) * 4],
                rs2_sb[:, NFT + ft:NFT + ft + 1].to_broadcast((128, 4)))
        t1 = top_pool.tile([128, 4 * NFT], F32, tag="t1")
        t2 = top_pool.tile([128, 4 * NFT], F32, tag="t2")
        uu = top_pool.tile([128, 4 * NFT], F32R, tag="uu")
        vv = top_pool.tile([128, 4 * NFT], F32R, tag="vv")
        nc.vector.tensor_mul(t1[:], rrep[:], wca_sb[:])
        nc.vector.tensor_mul(t2[:], srep[:], wsa_sb[:])
        nc.vector.tensor_add(uu[:], t1[:], t2[:])
        nc.vector.tensor_mul(t1[:], srep[:], wca_sb[:])
        nc.vector.tensor_mul(t2[:], rrep[:], wsa_sb[:])
        nc.vector.tensor_sub(vv[:], t1[:], t2[:])
        nyqrow = top_pool.tile([1, 4], F32R, tag="nyq")
        nc.vector.tensor_copy(
            nyqrow[:], rs2_sb[0:1, 2 * NFT:2 * NFT + 1].to_broadcast((1, 4)))

        mv_ps = psum_def.tile([4, 512], F32, tag="small", bufs=2, name="mvps")
        for ft in range(NFT):
            cb_sb = trigd_pool.tile([128, 512], F32R, tag="trig")
            nc.sync.dma_start(cb_sb[:], v["cosB3"][ft])
            sb_sb = trigd_pool.tile([128, 512], F32R, tag="trig")
            nc.sync.dma_start(sb_sb[:], v["sinB3"][ft])
            nc.tensor.matmul(mv_ps[:], uu[:, ft * 4:(ft + 1) * 4], cb_sb[:],
                             start=(ft == 0), stop=False)
            nc.tensor.matmul(mv_ps[:], vv[:, ft * 4:(ft + 1) * 4], sb_sb[:],
                             start=False, stop=False)
        nc.tensor.matmul(mv_ps[:], nyqrow[:], altb_sb[:],
                         start=False, stop=True)

        mv4 = top_pool.tile([4, 512], F32, tag="mv4")
        nc.vector.tensor_copy(mv4[:], mv_ps[:])
        mv_sb = top_pool.tile([1, L], F32, tag="mv")
        nc.gpsimd.dma_start(
            v["mv_dram"].rearrange("o (a b) -> a (o b)", a=4), mv4[:])
        nc.gpsimd.dma_start(mv_sb[:], v["mv_dram"][:])

        # top-k round 1: top-8 values; the gather's first tap wave can
        # start on unnormalized exp weights (the 1/sum scale is applied at
        # the output activation), overlapping round 2 with PE work.  The
        # weight chain (sub/exp/esr) is issued ahead of FIND_INDEX8 and of
        # the VPT evacuations so no queue blocks it.
        vals16 = top_pool.tile([1, 16], F32, tag="vals")
        idx16 = top_pool.tile([1, 16], U32, tag="idx")
        mv_m = top_pool.tile([1, L], F32, tag="mvm")
        m1 = vals16[0:1, 0:8]
        m2 = vals16[0:1, 8:16]
        es = top_pool.tile([1, 18], F32, tag="es")
        esr = top_pool.tile([1, 18], F32R, tag="esr")
        wbs = top_pool.tile([128, 18], F32, tag="wbs")

        nc.vector.max(m1, mv_sb[:])
        nc.vector.tensor_sub(es[0:1, 0:8], m1,
                             vals16[0:1, 0:1].to_broadcast((1, 8)))
        nc.scalar.activation(es[0:1, 0:8], es[0:1, 0:8], AF.Exp)
        nc.vector.tensor_copy(esr[0:1, 0:8], es[0:1, 0:8])
        nc.vector.max_index(idx16[0:1, 0:8], m1, mv_sb[:])

        vpt_jtile(2)
        vpt_jtile(3, on_vector=True)

        wb1 = psum_def.tile([128, 8], F32, tag="small", bufs=2, name="wb1")
        nc.tensor.matmul(wb1[:], ones_sb[:], esr[0:1, 0:8],
                         start=True, stop=True)
        nc.scalar.copy(wbs[:, 0:8], wb1[:])
        for j in range(8):
            nc.scalar.mul(wI[:, j * 128:(j + 1) * 128], ident_sb[:],
                          wbs[:, j:j + 1])
        _, deltas1 = nc.values_load_multi_w_load_instructions(
            idx16[0:1, 0:8], engines=(mybir.EngineType.PE,),
            min_val=0, max_val=L - 1, skip_runtime_bounds_check=True)

        # top-k round 2 (runs on DVE while the PE does tap wave 1)
        nc.vector.match_replace(mv_m[:], m1, mv_sb[:], -1e30)
        nc.vector.max(m2, mv_m[:])
        nc.vector.max_index(idx16[0:1, 8:16], m2, mv_m[:])
        nc.vector.tensor_sub(es[0:1, 8:16], m2,
                             vals16[0:1, 0:1].to_broadcast((1, 8)))
        nc.scalar.activation(es[0:1, 8:16], es[0:1, 8:16], AF.Exp)
        nc.vector.memset(es[0:1, 15:16], 0.0)
        nc.vector.reduce_sum(es[0:1, 16:17], es[0:1, 0:16],
                             axis=mybir.AxisListType.X)
        nc.vector.memset(es[0:1, 17:18], 0.0)
        nc.vector.tensor_copy(esr[0:1, 8:18], es[0:1, 8:18])

    # =============== Stage F: gather (15 taps, two waves) + output =========
    grp_tiles = [(jt, nch) for jt in range(4) for nch in range(4)]
    groups = [grp_tiles[0:6], grp_tiles[6:12], grp_tiles[12:16]]
    with tc.tile_pool(name="outp", bufs=1) as out_pool:
        deltas2 = None
        for gi, grp in enumerate(groups):
            pss = []
            for (jt, nch) in grp:
                ps = psum_def.tile([128, 512], F32, tag="ps", bufs=6,
                                   name=f"pg{jt}_{nch}")
                pss.append(ps)
            for j in range(8):
                for ps, (jt, nch) in zip(pss, grp):
                    nc.tensor.matmul(
                        ps[:], wI[:, j * 128:(j + 1) * 128],
                        vpt2[jt][:, bass.ds(deltas1[j] + nch * 512, 512)],
                        start=(j == 0), stop=False)
            if gi == 0:
                # round-2 weights: broadcast + wI build + register loads
                wb2 = psum_def.tile([128, 10], F32, tag="small", bufs=2,
                                    name="wb2")
                nc.tensor.matmul(wb2[:], ones_sb[:], esr[0:1, 8:18],
                                 start=True, stop=True)
                nc.vector.tensor_copy(wbs[:, 8:18], wb2[:])
                nc.vector.reciprocal(inv_sb[:], wbs[:, 16:17])
                for j in range(8, NTAP):
                    nc.vector.tensor_scalar_mul(
                        wI[:, j * 128:(j + 1) * 128], ident_sb[:],
                        wbs[:, j:j + 1])
                _, deltas2 = nc.values_load_multi_w_load_instructions(
                    idx16[0:1, 8:NTAP], engines=(mybir.EngineType.PE,),
                    min_val=0, max_val=L - 1,
                    skip_runtime_bounds_check=True)
            for j in range(8, NTAP):
                for ps, (jt, nch) in zip(pss, grp):
                    nc.tensor.matmul(
                        ps[:], wI[:, j * 128:(j + 1) * 128],
                        vpt2[jt][:, bass.ds(deltas2[j - 8] + nch * 512, 512)],
                        start=False, stop=(j == NTAP - 1))
            for ps, (jt, nch) in zip(pss, grp):
                o = out_pool.tile([128, 512], BF16, tag="oev", bufs=6)
                nc.scalar.activation(o[:], ps[:], AF.Identity,
                                     bias=bo2_sb[:, jt:jt + 1],
                                     scale=inv_sb[:])
                eng = nc.sync if (jt * 4 + nch) % 2 == 0 else nc.scalar
                eng.dma_start(
                    v["out_t"][jt * 128:(jt + 1) * 128,
                               nch * 512:(nch + 1) * 512], o[:])

    stack.close()


def _get_program():
    if "nc" not in _cache:
        _cache["nc"] = _build_program()
    return _cache["nc"]


def _fold(x):
    """x: [D, L] fp32 -> (x+, x-, x0col) folded per DFT even/odd symmetry."""
    xp = np.empty((D, 1024), np.float32)
    xm = np.empty((D, 1024), np.float32)
    xp[:, :1023] = x[:, 1:1024] + x[:, 2047:1024:-1]
    xm[:, :1023] = x[:, 1:1024] - x[:, 2047:1024:-1]
    xp[:, 1023] = x[:, 1024]
    xm[:, 1023] = 0.0
    x0 = np.ascontiguousarray(x[:, 0].reshape(NDT, 128).T)   # [128, 8]
    return xp.astype(NPBF16), xm.astype(NPBF16), x0.astype(NPBF16)


def kernel(queries, keys, values, Wq, bq, Wk, bk, Wv, bv, Wo, bo):
    queries = np.asarray(queries, np.float32)
    keys = np.asarray(keys, np.float32)
    values = np.asarray(values, np.float32)
    Wq = np.asarray(Wq, np.float32); bq = np.asarray(bq, np.float32)
    Wk = np.asarray(Wk, np.float32); bk = np.asarray(bk, np.float32)
    Wv = np.asarray(Wv, np.float32); bv = np.asarray(bv, np.float32)
    Wo = np.asarray(Wo, np.float32); bo = np.asarray(bo, np.float32)

    (cosF8, sinF8, altcol, wcaP, wsaP, cosB3, sinB3,
     altb_row) = _cache.setdefault("const", _host_constants())
    ones_row = np.ones((1, 128), np.float32)
    ident = np.eye(128, dtype=np.float32).astype(NPBF16)
    wvt = np.ascontiguousarray(Wv.T).astype(NPBF16)
    bv_dt = np.ascontiguousarray(bv.reshape(NDT, 128).T).astype(NPBF16)

    per_batch = []
    for b in range(B):
        xq = np.ascontiguousarray(queries[b].T)
        xk = np.ascontiguousarray(keys[b].T)
        xtv = np.ascontiguousarray(values[b].T).astype(NPBF16)
        per_batch.append((_fold(xq), _fold(xk), xtv))

    in_maps = []
    for core in range(N_CORES):
        b, half = core // 2, core % 2
        cs = slice(half * CH, (half + 1) * CH)
        (xqp, xqm, xq0), (xkp, xkm, xk0), xtv = per_batch[b]
        in_maps.append({
            "xqp": xqp, "xqm": xqm, "xq0": xq0,
            "xkp": xkp, "xkm": xkm, "xk0": xk0,
            "xtv": xtv,
            "wq": np.ascontiguousarray(Wq[:, cs]).astype(NPBF16),
            "wk": np.ascontiguousarray(Wk[:, cs]).astype(NPBF16),
            "wvt": wvt,
            "wo": np.ascontiguousarray(Wo[:, cs]).astype(NPBF16),
            "bv_dt": bv_dt,
            "bqL_row": np.ascontiguousarray((L * bq[cs])[None, :]),
            "bkL_row": np.ascontiguousarray((L * bk[cs])[None, :]),
            "bo_cols": np.ascontiguousarray(bo[cs].reshape(4, 128).T),
            "cosF8": cosF8, "sinF8": sinF8, "altcol": altcol,
            "cosB3": cosB3, "sinB3": sinB3,
            "wcaP": wcaP, "wsaP": wsaP, "altb_row": altb_row,
            "ones_row": ones_row, "ident": ident,
        })

    nc = _get_program()
    res = run_bass_kernel_spmd(nc, in_maps, core_ids=list(range(N_CORES)),
                               **_cache.get("run_kwargs", {}))
    _cache["last_result"] = res

    out = np.empty((B, L, D), np.float32)
    for core in range(N_CORES):
        b, half = core // 2, core % 2
        out[b, :, half * CH:(half + 1) * CH] = \
            res.results[core]["out_t"].T.astype(np.float32)
    return out


# revision 17
# speedup vs baseline: 1.0039x; 1.0039x over previous
"""Trainium2 Bass kernel for nn_CorrLayer (Autoformer AutoCorrelation layer).

Contract: kernel(**inputs) takes FULL inputs (queries/keys/values [4,2048,1024],
Wq/bq/Wk/bk/Wv/bv/Wo/bo) and returns the FULL output [4,2048,1024], running the
compute on 8 NeuronCores.

Sharding: core = 2*b + half.  Each core-pair handles one batch b:
  - q/k projections + DFT products are split by channel half (c-split);
    the per-frequency channel-sums R,S are all-reduced pairwise (8.7 KB).
  - the output projection + time-delay gather are split by output-column half.

Device algorithm (per core), matmul operands mostly bf16 (PSUM accum fp32):
  1. Host folds inputs by the DFT even/odd symmetry: for l'=1..1023,
     x+[l'] = x[l'] + x[2048-l'], x-[l'] = x[l'] - x[2048-l']; x+[1024] =
     x[1024]; plus the l=0 column.  This halves the DFT contraction length.
  2. q+/q-/k+/k- = projections of folded inputs ([l',c] tiles); q0/k0 rows.
  3. DFT-as-matmul on folded data, 8 f-chunks of 128 (f=0..1023):
     A = cosF^T q+ (+ q0 broadcast, + L*bq at f=0 only), As = sinF^T q-;
     R[f] = sum_c A*C, S[f] = sum_c As*Cs.  Nyquist bin f=1024 via the
     alternating-sign column (exact).
  4. Pairwise AllReduce of (R,S) [128,17].
  5. Wfused = Wv @ Wo[:,half] on device (bf16), so the value path needs no
     separate v projection: VPT[j,l] = Wfused^T xv^T directly (the duplicated
     full-channel v-projection and its DRAM spill are gone).
  6. mean corr mv[l] via factored irfft as one [4,512] PSUM matmul chain.
  7. top-16 of mv via two max8 rounds; softmax over top-15.
  8. out^T[j,l] = sum_k w_k VPT2[j, l+delta_k] via PSUM-accumulated
     scaled-identity matmuls with register-offset dynamic slices; + bo + bv@Wo.
Host: input transposes + folds, DFT constant matrices, output assembly.
"""
import math
import numpy as np
import ml_dtypes

import concourse.bass as bass
import concourse.bacc as bacc
import concourse.mybir as mybir
import concourse.tile as tile
from concourse.bass_utils import run_bass_kernel_spmd

F32 = mybir.dt.float32
F32R = mybir.dt.float32r
BF16 = mybir.dt.bfloat16
U32 = mybir.dt.uint32
AF = mybir.ActivationFunctionType
NPBF16 = ml_dtypes.bfloat16

B, L, D = 4, 2048, 1024
H, DK = 16, 64
CH = 512            # channels per core (c-split half)
NFT = 8             # f chunks of 128 -> bins 0..1023; Nyquist 1024 separate
NLT = 8             # l' tiles (l' = 1..1024 folded)
NDT = D // 128      # 8 d-tiles
TOPK = 15           # int(2*log(2048))
NTAP = 15
N_CORES = 8

_cache = {}


def _host_constants():
    f = np.arange(1024)
    lp = np.arange(1, 1025)                  # l' = 1..1024, j = l'-1
    ang = 2.0 * np.pi * np.outer(lp, f) / L
    cosF = np.cos(ang)                       # [1024 j, 1024 f]
    sinF = np.sin(ang)
    # SBUF chunk layout [ft, p, lt*128+fc] with p = j%128, lt = j//128
    def chunkify(m):
        return np.ascontiguousarray(
            m.reshape(NLT, 128, NFT, 128).transpose(2, 1, 0, 3)
            .reshape(NFT, 128, NLT * 128).astype(NPBF16))
    cosF8 = chunkify(cosF)
    sinF8 = chunkify(sinF)
    altcol = np.ascontiguousarray(
        ((-1.0) ** lp).reshape(NLT, 128).T.astype(NPBF16))   # [128, 8]

    # irfft, factored over l = a*512 + b:
    #   mv[a*512+b] = sum_f U[f,a] cosB[f,b] + V[f,a] sinB[f,b]
    #   U = wf(R cosA + S sinA), V = wf(S cosA - R sinA)
    wf = np.full(1025, 2.0 / L, np.float64)
    wf[0] = 1.0 / L
    wf[1024] = 1.0 / L
    wf = wf / (H * DK)   # fold the channel-mean into the inverse transform
    a4 = np.arange(4)
    b512 = np.arange(512)
    cosA = np.cos(np.pi * np.outer(f, a4) / 2.0)
    sinA = np.sin(np.pi * np.outer(f, a4) / 2.0)
    # [128 p, 32] with col = ft*4 + a
    wca = (wf[:1024, None] * cosA).astype(np.float32)
    wsa = (wf[:1024, None] * sinA).astype(np.float32)
    wcaP = np.ascontiguousarray(
        wca.reshape(NFT, 128, 4).transpose(1, 0, 2).reshape(128, NFT * 4))
    wsaP = np.ascontiguousarray(
        wsa.reshape(NFT, 128, 4).transpose(1, 0, 2).reshape(128, NFT * 4))
    cosB3 = np.ascontiguousarray(
        np.cos(2.0 * np.pi * np.outer(f, b512) / L)
        .astype(np.float32).reshape(NFT, 128, 512))
    sinB3 = np.ascontiguousarray(
        np.sin(2.0 * np.pi * np.outer(f, b512) / L)
        .astype(np.float32).reshape(NFT, 128, 512))
    altb_row = np.ascontiguousarray(
        (wf[1024] * ((-1.0) ** b512)).astype(np.float32)[None, :])  # [1,512]
    return cosF8, sinF8, altcol, wcaP, wsaP, cosB3, sinB3, altb_row


def _build_program():
    nc = bacc.Bacc("TRN2", target_bir_lowering=False, debug=False,
                   enable_asserts=False, num_devices=N_CORES)

    def din(name, shape, dt):
        return nc.dram_tensor(name, shape, dt, kind="ExternalInput").ap()

    v = {}
    for nm in ("xqp", "xqm", "xkp", "xkm"):
        v[nm] = din(nm, [D, 1024], BF16)
    v["xq0"] = din("xq0", [128, NDT], BF16)
    v["xk0"] = din("xk0", [128, NDT], BF16)
    v["xtv"] = din("xtv", [D, L], BF16)
    v["wq"] = din("wq", [D, CH], BF16)
    v["wk"] = din("wk", [D, CH], BF16)
    v["wvt"] = din("wvt", [D, D], BF16)
    v["wo"] = din("wo", [D, CH], BF16)
    v["bv_dt"] = din("bv_dt", [128, NDT], BF16)
    v["bqL_row"] = din("bqL_row", [1, CH], F32)
    v["bkL_row"] = din("bkL_row", [1, CH], F32)
    v["bo_cols"] = din("bo_cols", [128, 4], F32)
    v["cosF8"] = din("cosF8", [NFT, 128, NLT * 128], BF16)
    v["sinF8"] = din("sinF8", [NFT, 128, NLT * 128], BF16)
    v["altcol"] = din("altcol", [128, NLT], BF16)
    v["cosB3"] = din("cosB3", [NFT, 128, 512], F32R)
    v["sinB3"] = din("sinB3", [NFT, 128, 512], F32R)
    v["wcaP"] = din("wcaP", [128, 4 * NFT], F32)
    v["wsaP"] = din("wsaP", [128, 4 * NFT], F32)
    v["altb_row"] = din("altb_row", [1, 512], F32R)
    v["ones_row"] = din("ones_row", [1, 128], F32R)
    v["ident"] = din("ident", [128, 128], BF16)
    v["out_t"] = nc.dram_tensor("out_t", [CH, L], BF16,
                                kind="ExternalOutput").ap()

    with tile.TileContext(nc) as tc:
        with tc.tile_pool(name="dram", bufs=1, space="DRAM") as dram_pool:
            v["rs_in"] = dram_pool.tile([128, 2 * NFT + 1], F32,
                                        name="rs_in")
            v["rs_out"] = dram_pool.tile([128, 2 * NFT + 1], F32,
                                         name="rs_out")
            v["delta_dram"] = dram_pool.tile([1, CH], F32, name="delta_dram")
            v["mv_dram"] = dram_pool.tile([1, L], F32, name="mv_dram")
            _build_body(nc, tc, v)
    nc.compile()
    return nc


def _build_body(nc, tc, v):
    from contextlib import ExitStack
    stack = ExitStack()

    const_pool = stack.enter_context(tc.tile_pool(name="const", bufs=1))
    xq0_sb = const_pool.tile([128, NDT], BF16, tag="x0", bufs=2)
    nc.scalar.dma_start(xq0_sb[:], v["xq0"])
    xk0_sb = const_pool.tile([128, NDT], BF16, tag="x0", bufs=2)
    nc.scalar.dma_start(xk0_sb[:], v["xk0"])
    ones_sb = const_pool.tile([1, 128], F32R, tag="ones")
    nc.scalar.dma_start(ones_sb[:], v["ones_row"])
    ident_sb = const_pool.tile([128, 128], BF16, tag="ident")
    nc.scalar.dma_start(ident_sb[:], v["ident"])
    bo_sb = const_pool.tile([128, 4], F32, tag="bo")
    nc.scalar.dma_start(bo_sb[:], v["bo_cols"])
    bqL_sb = const_pool.tile([1, CH], F32, tag="bql")
    nc.scalar.dma_start(bqL_sb[:], v["bqL_row"])
    bkL_sb = const_pool.tile([1, CH], F32, tag="bkl")
    nc.scalar.dma_start(bkL_sb[:], v["bkL_row"])
    altcol_sb = const_pool.tile([128, NLT], BF16, tag="altc")
    nc.scalar.dma_start(altcol_sb[:], v["altcol"])
    bv_sb = const_pool.tile([128, NDT], BF16, tag="bv")
    nc.scalar.dma_start(bv_sb[:], v["bv_dt"])
    rs_sb = const_pool.tile([128, 2 * NFT + 1], F32, tag="rs")
    rs2_sb = const_pool.tile([128, 2 * NFT + 1], F32, tag="rs2")
    q0row = const_pool.tile([1, CH], F32R, tag="q0r", bufs=2)
    k0row = const_pool.tile([1, CH], F32R, tag="q0r", bufs=2)
    q0bc = const_pool.tile([128, CH], F32, tag="q0bc", bufs=3)
    k0bc = const_pool.tile([128, CH], F32, tag="q0bc", bufs=3)
    q0bc0 = const_pool.tile([128, CH], F32, tag="q0bc", bufs=3)
    wI = const_pool.tile([128, NTAP * 128], BF16, tag="wI")
    bo2_sb = const_pool.tile([128, 4], F32, tag="bo2")
    inv_sb = const_pool.tile([128, 1], F32, tag="inv")
    wca_sb = const_pool.tile([128, 4 * NFT], F32, tag="wca")
    nc.scalar.dma_start(wca_sb[:], v["wcaP"])
    wsa_sb = const_pool.tile([128, 4 * NFT], F32, tag="wsa")
    nc.scalar.dma_start(wsa_sb[:], v["wsaP"])
    altb_sb = const_pool.tile([1, 512], F32R, tag="altb")
    nc.scalar.dma_start(altb_sb[:], v["altb_row"])

    # =============== Stage A: folded projections ===============
    qp_sb = [None] * NLT
    qm_sb = [None] * NLT
    kp_sb = [None] * NLT
    km_sb = [None] * NLT
    qk_pool = stack.enter_context(tc.tile_pool(name="qk", bufs=4 * NLT))
    with tc.tile_pool(name="xin", bufs=9) as xin_pool, \
         tc.tile_pool(name="wqk", bufs=1) as w_pool, \
         tc.tile_pool(name="psumA", bufs=8, space="PSUM") as psum_a:
        wq_t, wk_t = [], []
        qp_x = []
        for dt_i in range(NDT):
            wt = w_pool.tile([128, CH], BF16, tag="wq", bufs=NDT,
                             name=f"wq{dt_i}")
            nc.sync.dma_start(wt[:], v["wq"][dt_i * 128:(dt_i + 1) * 128, :])
            wq_t.append(wt)
            xt_ = xin_pool.tile([128, 1024], BF16, tag="xin",
                                name=f"qpx{dt_i}")
            eng = nc.sync if dt_i % 2 == 0 else nc.scalar
            eng.dma_start(xt_[:], v["xqp"][dt_i * 128:(dt_i + 1) * 128, :])
            qp_x.append(xt_)
            wt = w_pool.tile([128, CH], BF16, tag="wk", bufs=NDT,
                             name=f"wk{dt_i}")
            nc.scalar.dma_start(wt[:], v["wk"][dt_i * 128:(dt_i + 1) * 128, :])
            wk_t.append(wt)
        for name, src, w_t, dst in (("qp", "xqp", wq_t, qp_sb),
                                    ("qm", "xqm", wq_t, qm_sb),
                                    ("kp", "xkp", wk_t, kp_sb),
                                    ("km", "xkm", wk_t, km_sb)):
            if name == "qp":
                x_t = qp_x
            else:
                x_t = []
                for dt_i in range(NDT):
                    xt_ = xin_pool.tile([128, 1024], BF16, tag="xin",
                                        name=f"{name}x{dt_i}")
                    eng = nc.sync if dt_i % 2 == 0 else nc.scalar
                    eng.dma_start(xt_[:],
                                  v[src][dt_i * 128:(dt_i + 1) * 128, :])
                    x_t.append(xt_)
            for ltg in range(2):
                ps = [psum_a.tile([128, CH], F32, tag="ps", bufs=6,
                                  name=f"p{name}{ltg}_{i}") for i in range(4)]
                for dt_i in range(NDT):
                    for li in range(4):
                        lt = ltg * 4 + li
                        nc.tensor.matmul(
                            ps[li][:],
                            x_t[dt_i][:, lt * 128:(lt + 1) * 128],
                            w_t[dt_i][:], start=(dt_i == 0),
                            stop=(dt_i == NDT - 1))
                for li in range(4):
                    t = qk_pool.tile([128, CH], BF16, tag="qk",
                                     name=f"{name}{ltg * 4 + li}")
                    nc.scalar.copy(t[:], ps[li][:])
                    dst[ltg * 4 + li] = t

        # q0/k0 rows (raw l=0 projections, no bias) + broadcast tiles
        ps_q0 = psum_a.tile([1, CH], F32, tag="ps0", bufs=2)
        ps_k0 = psum_a.tile([1, CH], F32, tag="ps0", bufs=2)
        for dt_i in range(NDT):
            nc.tensor.matmul(ps_q0[:], xq0_sb[:, dt_i:dt_i + 1], wq_t[dt_i][:],
                             start=(dt_i == 0), stop=(dt_i == NDT - 1))
        for dt_i in range(NDT):
            nc.tensor.matmul(ps_k0[:], xk0_sb[:, dt_i:dt_i + 1], wk_t[dt_i][:],
                             start=(dt_i == 0), stop=(dt_i == NDT - 1))
        nc.scalar.copy(q0row[:], ps_q0[:])
        nc.scalar.copy(k0row[:], ps_k0[:])
        ps_qb = psum_a.tile([128, CH], F32, tag="ps", bufs=6, name="psqb")
        ps_kb = psum_a.tile([128, CH], F32, tag="ps", bufs=6, name="pskb")
        nc.tensor.matmul(ps_qb[:], ones_sb[:], q0row[:], start=True, stop=True)
        nc.tensor.matmul(ps_kb[:], ones_sb[:], k0row[:], start=True, stop=True)
        nc.scalar.copy(q0bc[:], ps_qb[:])
        nc.scalar.copy(k0bc[:], ps_kb[:])
        # chunk-0 correction tile: q0bc + L*bq on the f=0 partition row
        nc.vector.tensor_copy(q0bc0[:], q0bc[:])
        nc.vector.tensor_add(q0bc0[0:1, :], q0bc[0:1, :], bqL_sb[:])

    # =============== Stage B: folded DFT + channel-summed products =========
    # trig chunks 0/1 prefetch first, then the value-path weights/inputs so
    # they sit ahead of the slot-blocked later trig chunks in the DMA queues
    trig_pool = stack.enter_context(tc.tile_pool(name="trigB", bufs=6))
    trig_tiles = {}
    for ft in range(2):
        cos_sb = trig_pool.tile([128, NLT * 128], BF16, tag="trig",
                                name=f"cos{ft}")
        nc.sync.dma_start(cos_sb[:], v["cosF8"][ft])
        sin_sb = trig_pool.tile([128, NLT * 128], BF16, tag="trig",
                                name=f"sin{ft}")
        nc.sync.dma_start(sin_sb[:], v["sinF8"][ft])
        trig_tiles[ft] = (cos_sb, sin_sb)
    wf_pool = stack.enter_context(tc.tile_pool(name="wf", bufs=1))
    xtv_pool = stack.enter_context(tc.tile_pool(name="xtv", bufs=1))
    wvt_t, wo_t, xtv_t, wf_t = [], [], [], []
    for ct in range(NDT):
        t = wf_pool.tile([128, D], BF16, tag="wvt", bufs=NDT, name=f"wvt{ct}")
        nc.sync.dma_start(t[:], v["wvt"][ct * 128:(ct + 1) * 128, :])
        wvt_t.append(t)
        t = wf_pool.tile([128, CH], BF16, tag="wo", bufs=NDT, name=f"wo{ct}")
        nc.scalar.dma_start(t[:], v["wo"][ct * 128:(ct + 1) * 128, :])
        wo_t.append(t)
        t = xtv_pool.tile([128, L], BF16, tag="xtv", bufs=NDT, name=f"xtv{ct}")
        eng = nc.sync if ct % 2 == 0 else nc.scalar
        eng.dma_start(t[:], v["xtv"][ct * 128:(ct + 1) * 128, :])
        xtv_t.append(t)

    with tc.tile_pool(name="ev", bufs=1) as ev_pool, \
         tc.tile_pool(name="psumB", bufs=8, space="PSUM") as psum_b:
        # Nyquist bin f=1024 (exact) first: its R lands in rs_sb early so the
        # collective fire is gated only by the last f-chunk
        pN1 = psum_b.tile([1, CH], F32, tag="psn", bufs=2, name="pN1")
        pN2 = psum_b.tile([1, CH], F32, tag="psn", bufs=2, name="pN2")
        for lt in range(NLT):
            st, sp = (lt == 0), (lt == NLT - 1)
            nc.tensor.matmul(pN1[:], altcol_sb[:, lt:lt + 1], qp_sb[lt][:],
                             start=st, stop=sp)
        for lt in range(NLT):
            st, sp = (lt == 0), (lt == NLT - 1)
            nc.tensor.matmul(pN2[:], altcol_sb[:, lt:lt + 1], kp_sb[lt][:],
                             start=st, stop=sp)
        eN1 = ev_pool.tile([1, CH], F32, tag="en", bufs=4)
        eN2 = ev_pool.tile([1, CH], F32, tag="en", bufs=4)
        nc.scalar.copy(eN1[:], pN1[:])
        nc.scalar.copy(eN2[:], pN2[:])
        eN1b = ev_pool.tile([1, CH], F32, tag="en", bufs=4)
        eN2b = ev_pool.tile([1, CH], F32, tag="en", bufs=4)
        nc.vector.tensor_add(eN1b[:], eN1[:], q0row[:])
        nc.vector.tensor_add(eN2b[:], eN2[:], k0row[:])
        prodN = ev_pool.tile([1, CH], F32, tag="pn")
        nc.vector.tensor_mul(prodN[:], eN1b[:], eN2b[:])
        nc.vector.memset(rs_sb[:, 2 * NFT:2 * NFT + 1], 0.0)
        nc.vector.reduce_sum(rs_sb[0:1, 2 * NFT:2 * NFT + 1], prodN[:],
                             axis=mybir.AxisListType.X)

        for ft in range(NFT):
            if ft < 2:
                cos_sb, sin_sb = trig_tiles[ft]
            else:
                cos_sb = trig_pool.tile([128, NLT * 128], BF16, tag="trig",
                                        name=f"cos{ft}")
                nc.sync.dma_start(cos_sb[:], v["cosF8"][ft])
                sin_sb = trig_pool.tile([128, NLT * 128], BF16, tag="trig",
                                        name=f"sin{ft}")
                nc.sync.dma_start(sin_sb[:], v["sinF8"][ft])

            pA = psum_b.tile([128, CH], F32, tag="ps", bufs=6, name="pA")
            pC = psum_b.tile([128, CH], F32, tag="ps", bufs=6, name="pC")
            pAs = psum_b.tile([128, CH], F32, tag="ps", bufs=6, name="pAs")
            pCs = psum_b.tile([128, CH], F32, tag="ps", bufs=6, name="pCs")
            for lt in range(NLT):
                st, sp = (lt == 0), (lt == NLT - 1)
                cs = cos_sb[:, lt * 128:(lt + 1) * 128]
                ss = sin_sb[:, lt * 128:(lt + 1) * 128]
                nc.tensor.matmul(pA[:], cs, qp_sb[lt][:], start=st, stop=sp)
                nc.tensor.matmul(pC[:], cs, kp_sb[lt][:], start=st, stop=sp)
                nc.tensor.matmul(pAs[:], ss, qm_sb[lt][:], start=st, stop=sp)
                nc.tensor.matmul(pCs[:], ss, km_sb[lt][:], start=st, stop=sp)

            eA = ev_pool.tile([128, CH], F32, tag="ev", bufs=6)
            eC = ev_pool.tile([128, CH], F32, tag="ev", bufs=6)
            eAs = ev_pool.tile([128, CH], F32, tag="ev", bufs=6)
            eCs = ev_pool.tile([128, CH], F32, tag="ev", bufs=6)
            nc.scalar.copy(eA[:], pA[:])
            nc.scalar.copy(eC[:], pC[:])
            nc.scalar.copy(eAs[:], pAs[:])
            nc.scalar.copy(eCs[:], pCs[:])
            eA2 = ev_pool.tile([128, CH], F32, tag="ev2", bufs=4)
            eC2 = ev_pool.tile([128, CH], F32, tag="ev2", bufs=4)
            nc.vector.tensor_add(eA2[:], eA[:], (q0bc0 if ft == 0 else q0bc)[:])
            nc.vector.tensor_add(eC2[:], eC[:], k0bc[:])
            prodR = ev_pool.tile([128, CH], F32, tag="prod", bufs=2)
            prodS = ev_pool.tile([128, CH], F32, tag="prod", bufs=2)
            nc.vector.tensor_mul(prodR[:], eA2[:], eC2[:])
            nc.vector.tensor_mul(prodS[:], eAs[:], eCs[:])
            nc.vector.reduce_sum(rs_sb[:, ft:ft + 1], prodR[:],
                                 axis=mybir.AxisListType.X)
            nc.vector.reduce_sum(rs_sb[:, NFT + ft:NFT + ft + 1], prodS[:],
                                 axis=mybir.AxisListType.X)

    # =============== Stage C: pairwise all-reduce of (R,S) ===============
    nc.gpsimd.dma_start(v["rs_in"][:], rs_sb[:])
    nc.gpsimd.collective_compute(
        "AllReduce", mybir.AluOpType.add,
        replica_groups=[[0, 1], [2, 3], [4, 5], [6, 7]],
        ins=[v["rs_in"].opt()], outs=[v["rs_out"].opt()])
    nc.gpsimd.dma_start(rs2_sb[:], v["rs_out"][:])

    psum_def = stack.enter_context(
        tc.tile_pool(name="psumDEF", bufs=8, space="PSUM"))
    vpt_pool = stack.enter_context(tc.tile_pool(name="vpt", bufs=1))
    vpt2 = [vpt_pool.tile([128, 2 * L], BF16, tag="vpt2", bufs=4,
                          name=f"vpt2_{i}") for i in range(4)]

    # =============== Stage W: fused Wv@Wo + value transform ===============
    # (issued after the collective fire so the PE chews it during the
    #  allreduce + reload window)
    for dt_i in range(NDT):
        ps = psum_def.tile([128, CH], F32, tag="ps", bufs=6, name=f"pwf{dt_i}")
        for ct in range(NDT):
            nc.tensor.matmul(ps[:], wvt_t[ct][:, dt_i * 128:(dt_i + 1) * 128],
                             wo_t[ct][:], start=(ct == 0), stop=(ct == NDT - 1))
        t = wf_pool.tile([128, CH], BF16, tag="wf", bufs=NDT, name=f"wf{dt_i}")
        nc.scalar.copy(t[:], ps[:])
        wf_t.append(t)
    # delta row = bv @ Wo_half -> [1,512] -> DRAM -> [128,4] -> bo2
    ps_d = psum_def.tile([1, CH], F32, tag="small", bufs=2, name="ps_d")
    for ct in range(NDT):
        nc.tensor.matmul(ps_d[:], bv_sb[:, ct:ct + 1], wo_t[ct][:],
                         start=(ct == 0), stop=(ct == NDT - 1))
    drow = const_pool.tile([1, CH], F32, tag="drow")
    nc.scalar.copy(drow[:], ps_d[:])
    nc.scalar.dma_start(v["delta_dram"][:], drow[:])
    dcols = const_pool.tile([128, 4], F32, tag="dcols")
    nc.scalar.dma_start(
        dcols[:], v["delta_dram"].rearrange("o (a p) -> p (o a)", p=128))
    nc.vector.tensor_add(bo2_sb[:], bo_sb[:], dcols[:])

    def vpt_jtile(jt, on_vector=False):
        for lch in range(4):
            ps = psum_def.tile([128, 512], F32, tag="ps", bufs=6, name=f"pv{jt}_{lch}")
            for dt_i in range(NDT):
                nc.tensor.matmul(
                    ps[:], wf_t[dt_i][:, jt * 128:(jt + 1) * 128],
                    xtv_t[dt_i][:, lch * 512:(lch + 1) * 512],
                    start=(dt_i == 0), stop=(dt_i == NDT - 1))
            if on_vector:
                nc.vector.tensor_copy(vpt2[jt][:, lch * 512:(lch + 1) * 512],
                                      ps[:])
            else:
                nc.scalar.copy(vpt2[jt][:, lch * 512:(lch + 1) * 512], ps[:])
            eng = nc.sync if lch % 2 == 0 else nc.scalar
            eng.dma_start(vpt2[jt][:, L + lch * 512:L + (lch + 1) * 512],
                          vpt2[jt][:, lch * 512:(lch + 1) * 512])

    vpt_jtile(0)
    vpt_jtile(1)

    # =============== Stage D: irfft (factored) + top-16 + weights ==========
    with tc.tile_pool(name="trigD", bufs=4) as trigd_pool, \
         tc.tile_pool(name="top", bufs=1) as top_pool:
        rrep = top_pool.tile([128, 4 * NFT], F32, tag="rrep")
        srep = top_pool.tile([128, 4 * NFT], F32, tag="srep")
        for ft in range(NFT):
            nc.vector.tensor_copy(
                rrep[:, ft * 4:(ft + 1) * 4],
                rs2_sb[:, ft:ft + 1].to_broadcast((128, 4)))
            nc.vector.tensor_copy(
                srep[:, ft * 4:(ft + 1) * 4],
                rs2_sb[:, NFT + ft:NFT + ft + 1].to_broadcast((128, 4)))
        t1 = top_pool.tile([128, 4 * NFT], F32, tag="t1")
        t2 = top_pool.tile([128, 4 * NFT], F32, tag="t2")
        uu = top_pool.tile([128, 4 * NFT], F32R, tag="uu")
        vv = top_pool.tile([128, 4 * NFT], F32R, tag="vv")
        nc.vector.tensor_mul(t1[:], rrep[:], wca_sb[:])
        nc.vector.tensor_mul(t2[:], srep[:], wsa_sb[:])
        nc.vector.tensor_add(uu[:], t1[:], t2[:])
        nc.vector.tensor_mul(t1[:], srep[:], wca_sb[:])
        nc.vector.tensor_mul(t2[:], rrep[:], wsa_sb[:])
        nc.vector.tensor_sub(vv[:], t1[:], t2[:])
        nyqrow = top_pool.tile([1, 4], F32R, tag="nyq")
        nc.vector.tensor_copy(
            nyqrow[:], rs2_sb[0:1, 2 * NFT:2 * NFT + 1].to_broadcast((1, 4)))

        mv_ps = psum_def.tile([4, 512], F32, tag="small", bufs=2, name="mvps")
        for ft in range(NFT):
            cb_sb = trigd_pool.tile([128, 512], F32R, tag="trig")
            nc.sync.dma_start(cb_sb[:], v["cosB3"][ft])
            sb_sb = trigd_pool.tile([128, 512], F32R, tag="trig")
            nc.sync.dma_start(sb_sb[:], v["sinB3"][ft])
            nc.tensor.matmul(mv_ps[:], uu[:, ft * 4:(ft + 1) * 4], cb_sb[:],
                             start=(ft == 0), stop=False)
            nc.tensor.matmul(mv_ps[:], vv[:, ft * 4:(ft + 1) * 4], sb_sb[:],
                             start=False, stop=False)
        nc.tensor.matmul(mv_ps[:], nyqrow[:], altb_sb[:],
                         start=False, stop=True)

        mv4 = top_pool.tile([4, 512], F32, tag="mv4")
        nc.vector.tensor_copy(mv4[:], mv_ps[:])
        mv_sb = top_pool.tile([1, L], F32, tag="mv")
        nc.gpsimd.dma_start(
            v["mv_dram"].rearrange("o (a b) -> a (o b)", a=4), mv4[:])
        nc.gpsimd.dma_start(mv_sb[:], v["mv_dram"][:])

        # top-k round 1: top-8 values; the gather's first tap wave can
        # start on unnormalized exp weights (the 1/sum scale is applied at
        # the output activation), overlapping round 2 with PE work.  The
        # weight chain (sub/exp/esr) is issued ahead of FIND_INDEX8 and of
        # the VPT evacuations so no queue blocks it.
        vals16 = top_pool.tile([1, 16], F32, tag="vals")
        idx16 = top_pool.tile([1, 16], U32, tag="idx")
        mv_m = top_pool.tile([1, L], F32, tag="mvm")
        m1 = vals16[0:1, 0:8]
        m2 = vals16[0:1, 8:16]
        es = top_pool.tile([1, 18], F32, tag="es")
        esr = top_pool.tile([1, 18], F32R, tag="esr")
        wbs = top_pool.tile([128, 18], F32, tag="wbs")

        nc.vector.max(m1, mv_sb[:])
        nc.vector.tensor_sub(es[0:1, 0:8], m1,
                             vals16[0:1, 0:1].to_broadcast((1, 8)))
        nc.scalar.activation(es[0:1, 0:8], es[0:1, 0:8], AF.Exp)
        nc.vector.tensor_copy(esr[0:1, 0:8], es[0:1, 0:8])
        nc.vector.max_index(idx16[0:1, 0:8], m1, mv_sb[:])

        vpt_jtile(2)
        vpt_jtile(3, on_vector=True)

        wb1 = psum_def.tile([128, 8], F32, tag="small", bufs=2, name="wb1")
        nc.tensor.matmul(wb1[:], ones_sb[:], esr[0:1, 0:8],
                         start=True, stop=True)
        nc.scalar.copy(wbs[:, 0:8], wb1[:])
        for j in range(8):
            nc.scalar.mul(wI[:, j * 128:(j + 1) * 128], ident_sb[:],
                          wbs[:, j:j + 1])
        _, deltas1 = nc.values_load_multi_w_load_instructions(
            idx16[0:1, 0:8], engines=(mybir.EngineType.PE,),
            min_val=0, max_val=L - 1, skip_runtime_bounds_check=True)

        # top-k round 2 (runs on DVE while the PE does tap wave 1)
        nc.vector.match_replace(mv_m[:], m1, mv_sb[:], -1e30)
        nc.vector.max(m2, mv_m[:])
        nc.vector.max_index(idx16[0:1, 8:16], m2, mv_m[:])
        nc.vector.tensor_sub(es[0:1, 8:16], m2,
                             vals16[0:1, 0:1].to_broadcast((1, 8)))
        nc.scalar.activation(es[0:1, 8:16], es[0:1, 8:16], AF.Exp)
        nc.vector.memset(es[0:1, 15:16], 0.0)
        nc.vector.reduce_sum(es[0:1, 16:17], es[0:1, 0:16],
                             axis=mybir.AxisListType.X)
        nc.vector.memset(es[0:1, 17:18], 0.0)
        nc.vector.tensor_copy(esr[0:1, 8:18], es[0:1, 8:18])

    # =============== Stage F: gather (15 taps, two waves) + output =========
    grp_tiles = [(jt, nch) for jt in range(4) for nch in range(4)]
    groups = [grp_tiles[0:6], grp_tiles[6:12], grp_tiles[12:16]]
    with tc.tile_pool(name="outp", bufs=1) as out_pool:
        deltas2 = None
        for gi, grp in enumerate(groups):
            pss = []
            for (jt, nch) in grp:
                ps = psum_def.tile([128, 512], F32, tag="ps", bufs=6,
                                   name=f"pg{jt}_{nch}")
                pss.append(ps)
            for j in range(8):
                for ps, (jt, nch) in zip(pss, grp):
                    nc.tensor.matmul(
                        ps[:], wI[:, j * 128:(j + 1) * 128],
                        vpt2[jt][:, bass.ds(deltas1[j] + nch * 512, 512)],
                        start=(j == 0), stop=False)
            if gi == 0:
                # round-2 weights: broadcast + wI build + register loads
                wb2 = psum_def.tile([128, 10], F32, tag="small", bufs=2,
                                    name="wb2")
                nc.tensor.matmul(wb2[:], ones_sb[:], esr[0:1, 8:18],
                                 start=True, stop=True)
                nc.vector.tensor_copy(wbs[:, 8:18], wb2[:])
                nc.vector.reciprocal(inv_sb[:], wbs[:, 16:17])
                for j in range(8, NTAP):
                    nc.vector.tensor_scalar_mul(
                        wI[:, j * 128:(j + 1) * 128], ident_sb[:],
                        wbs[:, j:j + 1])
                _, deltas2 = nc.values_load_multi_w_load_instructions(
                    idx16[0:1, 8:NTAP], engines=(mybir.EngineType.PE,),
                    min_val=0, max_val=L - 1,
                    skip_runtime_bounds_check=True)
            for j in range(8, NTAP):
                for ps, (jt, nch) in zip(pss, grp):
                    nc.tensor.matmul(
                        ps[:], wI[:, j * 128:(j + 1) * 128],
                        vpt2[jt][:, bass.ds(deltas2[j - 8] + nch * 512, 512)],
                        start=False, stop=(j == NTAP - 1))
            for ps, (jt, nch) in zip(pss, grp):
                o = out_pool.tile([128, 512], BF16, tag="oev", bufs=6)
                nc.scalar.activation(o[:], ps[:], AF.Identity,
                                     bias=bo2_sb[:, jt:jt + 1],
                                     scale=inv_sb[:])
                eng = nc.sync if (jt * 4 + nch) % 2 == 0 else nc.scalar
                eng.dma_start(
                    v["out_t"][jt * 128:(jt + 1) * 128,
                               nch * 512:(nch + 1) * 512], o[:])

    stack.close()


def _get_program():
    if "nc" not in _cache:
        _cache["nc"] = _build_program()
    return _cache["nc"]


def _fold(x):
    """x: [D, L] fp32 -> (x+, x-, x0col) folded per DFT even/odd symmetry."""
    xp = np.empty((D, 1024), np.float32)
    xm = np.empty((D, 1024), np.float32)
    xp[:, :1023] = x[:, 1:1024] + x[:, 2047:1024:-1]
    xm[:, :1023] = x[:, 1:1024] - x[:, 2047:1024:-1]
    xp[:, 1023] = x[:, 1024]
    xm[:, 1023] = 0.0
    x0 = np.ascontiguousarray(x[:, 0].reshape(NDT, 128).T)   # [128, 8]
    return xp.astype(NPBF16), xm.astype(NPBF16), x0.astype(NPBF16)


def kernel(queries, keys, values, Wq, bq, Wk, bk, Wv, bv, Wo, bo):
    queries = np.asarray(queries, np.float32)
    keys = np.asarray(keys, np.float32)
    values = np.asarray(values, np.float32)
    Wq = np.asarray(Wq, np.float32); bq = np.asarray(bq, np.float32)
    Wk = np.asarray(Wk, np.float32); bk = np.asarray(bk, np.float32)
    Wv = np.asarray(Wv, np.float32); bv = np.asarray(bv, np.float32)
    Wo = np.asarray(Wo, np.float32); bo = np.asarray(bo, np.float32)

    (cosF8, sinF8, altcol, wcaP, wsaP, cosB3, sinB3,
     altb_row) = _cache.setdefault("const", _host_constants())
    ones_row = np.ones((1, 128), np.float32)
    ident = np.eye(128, dtype=np.float32).astype(NPBF16)
    wvt = np.ascontiguousarray(Wv.T).astype(NPBF16)
    bv_dt = np.ascontiguousarray(bv.reshape(NDT, 128).T).astype(NPBF16)

    per_batch = []
    for b in range(B):
        xq = np.ascontiguousarray(queries[b].T)
        xk = np.ascontiguousarray(keys[b].T)
        xtv = np.ascontiguousarray(values[b].T).astype(NPBF16)
        per_batch.append((_fold(xq), _fold(xk), xtv))

    in_maps = []
    for core in range(N_CORES):
        b, half = core // 2, core % 2
        cs = slice(half * CH, (half + 1) * CH)
        (xqp, xqm, xq0), (xkp, xkm, xk0), xtv = per_batch[b]
        in_maps.append({
            "xqp": xqp, "xqm": xqm, "xq0": xq0,
            "xkp": xkp, "xkm": xkm, "xk0": xk0,
            "xtv": xtv,
            "wq": np.ascontiguousarray(Wq[:, cs]).astype(NPBF16),
            "wk": np.ascontiguousarray(Wk[:, cs]).astype(NPBF16),
            "wvt": wvt,
            "wo": np.ascontiguousarray(Wo[:, cs]).astype(NPBF16),
            "bv_dt": bv_dt,
            "bqL_row": np.ascontiguousarray((L * bq[cs])[None, :]),
            "bkL_row": np.ascontiguousarray((L * bk[cs])[None, :]),
            "bo_cols": np.ascontiguousarray(bo[cs].reshape(4, 128).T),
            "cosF8": cosF8, "sinF8": sinF8, "altcol": altcol,
            "cosB3": cosB3, "sinB3": sinB3,
            "wcaP": wcaP, "wsaP": wsaP, "altb_row": altb_row,
            "ones_row": ones_row, "ident": ident,
        })

    nc = _get_program()
    res = run_bass_kernel_spmd(nc, in_maps, core_ids=list(range(N_CORES)),
                               **_cache.get("run_kwargs", {}))
    _cache["last_result"] = res

    out = np.empty((B, L, D), np.float32)
    for core in range(N_CORES):
        b, half = core // 2, core % 2
        out[b, :, half * CH:(half + 1) * CH] = \
            res.results[core]["out_t"].T.astype(np.float32)
    return out


# revision 18
# speedup vs baseline: 1.0173x; 1.0133x over previous
"""Trainium2 Bass kernel for nn_CorrLayer (Autoformer AutoCorrelation layer).

Contract: kernel(**inputs) takes FULL inputs (queries/keys/values [4,2048,1024],
Wq/bq/Wk/bk/Wv/bv/Wo/bo) and returns the FULL output [4,2048,1024], running the
compute on 8 NeuronCores.

Sharding: core = 2*b + half.  Each core-pair handles one batch b:
  - q/k projections + DFT products are split by channel half (c-split);
    the per-frequency channel-sums R,S are all-reduced pairwise (8.7 KB).
  - the output projection + time-delay gather are split by output-column half.

Device algorithm (per core), matmul operands mostly bf16 (PSUM accum fp32):
  1. Host folds inputs by the DFT even/odd symmetry: for l'=1..1023,
     x+[l'] = x[l'] + x[2048-l'], x-[l'] = x[l'] - x[2048-l']; x+[1024] =
     x[1024]; plus the l=0 column.  This halves the DFT contraction length.
  2. q+/q-/k+/k- = projections of folded inputs ([l',c] tiles); q0/k0 rows.
  3. DFT-as-matmul on folded data, 8 f-chunks of 128 (f=0..1023):
     A = cosF^T q+ (+ q0 broadcast, + L*bq at f=0 only), As = sinF^T q-;
     R[f] = sum_c A*C, S[f] = sum_c As*Cs.  Nyquist bin f=1024 via the
     alternating-sign column (exact).
  4. Pairwise AllReduce of (R,S) [128,17].
  5. Wfused = Wv @ Wo[:,half] on device (bf16), so the value path needs no
     separate v projection: VPT[j,l] = Wfused^T xv^T directly (the duplicated
     full-channel v-projection and its DRAM spill are gone).
  6. mean corr mv[l] via factored irfft as one [4,512] PSUM matmul chain.
  7. top-16 of mv via two max8 rounds; softmax over top-15.
  8. out^T[j,l] = sum_k w_k VPT2[j, l+delta_k] via PSUM-accumulated
     scaled-identity matmuls with register-offset dynamic slices; + bo + bv@Wo.
Host: input transposes + folds, DFT constant matrices, output assembly.
"""
import math
import numpy as np
import ml_dtypes

import concourse.bass as bass
import concourse.bacc as bacc
import concourse.mybir as mybir
import concourse.tile as tile
from concourse.bass_utils import run_bass_kernel_spmd

F32 = mybir.dt.float32
F32R = mybir.dt.float32r
BF16 = mybir.dt.bfloat16
U32 = mybir.dt.uint32
AF = mybir.ActivationFunctionType
NPBF16 = ml_dtypes.bfloat16

B, L, D = 4, 2048, 1024
H, DK = 16, 64
CH = 512            # channels per core (c-split half)
NFT = 8             # f chunks of 128 -> bins 0..1023; Nyquist 1024 separate
NLT = 8             # l' tiles (l' = 1..1024 folded)
NDT = D // 128      # 8 d-tiles
TOPK = 15           # int(2*log(2048))
NTAP = 15
N_CORES = 8

_cache = {}


def _host_constants():
    f = np.arange(1024)
    lp = np.arange(1, 1025)                  # l' = 1..1024, j = l'-1
    ang = 2.0 * np.pi * np.outer(lp, f) / L
    cosF = np.cos(ang)                       # [1024 j, 1024 f]
    sinF = np.sin(ang)
    # SBUF chunk layout [ft, p, lt*128+fc] with p = j%128, lt = j//128
    def chunkify(m):
        return np.ascontiguousarray(
            m.reshape(NLT, 128, NFT, 128).transpose(2, 1, 0, 3)
            .reshape(NFT, 128, NLT * 128).astype(NPBF16))
    cosF8 = chunkify(cosF)
    sinF8 = chunkify(sinF)
    altcol = np.ascontiguousarray(
        ((-1.0) ** lp).reshape(NLT, 128).T.astype(NPBF16))   # [128, 8]

    # irfft, factored over l = a*512 + b:
    #   mv[a*512+b] = sum_f U[f,a] cosB[f,b] + V[f,a] sinB[f,b]
    #   U = wf(R cosA + S sinA), V = wf(S cosA - R sinA)
    wf = np.full(1025, 2.0 / L, np.float64)
    wf[0] = 1.0 / L
    wf[1024] = 1.0 / L
    wf = wf / (H * DK)   # fold the channel-mean into the inverse transform
    a4 = np.arange(4)
    b512 = np.arange(512)
    cosA = np.cos(np.pi * np.outer(f, a4) / 2.0)
    sinA = np.sin(np.pi * np.outer(f, a4) / 2.0)
    # [128 p, 32] with col = ft*4 + a
    wca = (wf[:1024, None] * cosA).astype(np.float32)
    wsa = (wf[:1024, None] * sinA).astype(np.float32)
    wcaP = np.ascontiguousarray(
        wca.reshape(NFT, 128, 4).transpose(1, 0, 2).reshape(128, NFT * 4))
    wsaP = np.ascontiguousarray(
        wsa.reshape(NFT, 128, 4).transpose(1, 0, 2).reshape(128, NFT * 4))
    cosB3 = np.ascontiguousarray(
        np.cos(2.0 * np.pi * np.outer(f, b512) / L)
        .astype(np.float32).reshape(NFT, 128, 512))
    sinB3 = np.ascontiguousarray(
        np.sin(2.0 * np.pi * np.outer(f, b512) / L)
        .astype(np.float32).reshape(NFT, 128, 512))
    altb_row = np.ascontiguousarray(
        (wf[1024] * ((-1.0) ** b512)).astype(np.float32)[None, :])  # [1,512]
    return cosF8, sinF8, altcol, wcaP, wsaP, cosB3, sinB3, altb_row


def _build_program():
    nc = bacc.Bacc("TRN2", target_bir_lowering=False, debug=False,
                   enable_asserts=False, num_devices=N_CORES)

    def din(name, shape, dt):
        return nc.dram_tensor(name, shape, dt, kind="ExternalInput").ap()

    v = {}
    for nm in ("xqp", "xqm", "xkp", "xkm"):
        v[nm] = din(nm, [D, 1024], BF16)
    v["xq0"] = din("xq0", [128, NDT], BF16)
    v["xk0"] = din("xk0", [128, NDT], BF16)
    v["xtv"] = din("xtv", [D, L], BF16)
    v["wq"] = din("wq", [D, CH], BF16)
    v["wk"] = din("wk", [D, CH], BF16)
    v["wvt"] = din("wvt", [D, D], BF16)
    v["wo"] = din("wo", [D, CH], BF16)
    v["bv_dt"] = din("bv_dt", [128, NDT], BF16)
    v["bqL_row"] = din("bqL_row", [1, CH], F32)
    v["bkL_row"] = din("bkL_row", [1, CH], F32)
    v["bo_cols"] = din("bo_cols", [128, 4], F32)
    v["cosF8"] = din("cosF8", [NFT, 128, NLT * 128], BF16)
    v["sinF8"] = din("sinF8", [NFT, 128, NLT * 128], BF16)
    v["altcol"] = din("altcol", [128, NLT], BF16)
    v["cosB3"] = din("cosB3", [NFT, 128, 512], F32R)
    v["sinB3"] = din("sinB3", [NFT, 128, 512], F32R)
    v["wcaP"] = din("wcaP", [128, 4 * NFT], F32)
    v["wsaP"] = din("wsaP", [128, 4 * NFT], F32)
    v["altb_row"] = din("altb_row", [1, 512], F32R)
    v["ones_row"] = din("ones_row", [1, 128], F32R)
    v["ident"] = din("ident", [128, 128], BF16)
    v["out_t"] = nc.dram_tensor("out_t", [CH, L], BF16,
                                kind="ExternalOutput").ap()

    with tile.TileContext(nc) as tc:
        with tc.tile_pool(name="dram", bufs=1, space="DRAM") as dram_pool:
            v["rs_in"] = dram_pool.tile([128, 2 * NFT + 1], F32,
                                        name="rs_in")
            v["rs_out"] = dram_pool.tile([128, 2 * NFT + 1], F32,
                                         name="rs_out")
            v["delta_dram"] = dram_pool.tile([1, CH], F32, name="delta_dram")
            v["mv_dram"] = dram_pool.tile([1, L], F32, name="mv_dram")
            _build_body(nc, tc, v)
    nc.compile()
    return nc


def _build_body(nc, tc, v):
    from contextlib import ExitStack
    stack = ExitStack()

    const_pool = stack.enter_context(tc.tile_pool(name="const", bufs=1))
    ones_sb = const_pool.tile([1, 128], F32R, tag="ones")
    nc.scalar.dma_start(ones_sb[:], v["ones_row"])
    ident_sb = const_pool.tile([128, 128], BF16, tag="ident")
    nc.scalar.dma_start(ident_sb[:], v["ident"])
    bo_sb = const_pool.tile([128, 4], F32, tag="bo")
    nc.scalar.dma_start(bo_sb[:], v["bo_cols"])
    bqL_sb = const_pool.tile([1, CH], F32, tag="bql")
    nc.scalar.dma_start(bqL_sb[:], v["bqL_row"])
    bkL_sb = const_pool.tile([1, CH], F32, tag="bkl")
    nc.scalar.dma_start(bkL_sb[:], v["bkL_row"])
    altcol_sb = const_pool.tile([128, NLT], BF16, tag="altc")
    nc.scalar.dma_start(altcol_sb[:], v["altcol"])
    xq0_sb = const_pool.tile([128, NDT], BF16, tag="x0", bufs=2)
    nc.scalar.dma_start(xq0_sb[:], v["xq0"])
    xk0_sb = const_pool.tile([128, NDT], BF16, tag="x0", bufs=2)
    nc.scalar.dma_start(xk0_sb[:], v["xk0"])
    bv_sb = const_pool.tile([128, NDT], BF16, tag="bv")
    nc.scalar.dma_start(bv_sb[:], v["bv_dt"])
    rs_sb = const_pool.tile([128, 2 * NFT + 1], F32, tag="rs")
    rs2_sb = const_pool.tile([128, 2 * NFT + 1], F32, tag="rs2")
    q0row = const_pool.tile([1, CH], F32R, tag="q0r", bufs=2)
    k0row = const_pool.tile([1, CH], F32R, tag="q0r", bufs=2)
    q0bc = const_pool.tile([128, CH], F32, tag="q0bc", bufs=3)
    k0bc = const_pool.tile([128, CH], F32, tag="q0bc", bufs=3)
    q0bc0 = const_pool.tile([128, CH], F32, tag="q0bc", bufs=3)
    wI = const_pool.tile([128, NTAP * 128], BF16, tag="wI")
    bo2_sb = const_pool.tile([128, 4], F32, tag="bo2")
    inv_sb = const_pool.tile([128, 1], F32, tag="inv")
    wca_sb = const_pool.tile([128, 4 * NFT], F32, tag="wca")
    nc.scalar.dma_start(wca_sb[:], v["wcaP"])
    wsa_sb = const_pool.tile([128, 4 * NFT], F32, tag="wsa")
    nc.scalar.dma_start(wsa_sb[:], v["wsaP"])
    altb_sb = const_pool.tile([1, 512], F32R, tag="altb")
    nc.scalar.dma_start(altb_sb[:], v["altb_row"])

    # =============== Stage A: folded projections ===============
    qp_sb = [None] * NLT
    qm_sb = [None] * NLT
    kp_sb = [None] * NLT
    km_sb = [None] * NLT
    qk_pool = stack.enter_context(tc.tile_pool(name="qk", bufs=4 * NLT))
    with tc.tile_pool(name="xin", bufs=9) as xin_pool, \
         tc.tile_pool(name="wqk", bufs=1) as w_pool, \
         tc.tile_pool(name="psumA", bufs=8, space="PSUM") as psum_a:
        wq_t, wk_t = [], []
        qp_x = []
        for dt_i in range(NDT):
            wt = w_pool.tile([128, CH], BF16, tag="wq", bufs=NDT,
                             name=f"wq{dt_i}")
            nc.sync.dma_start(wt[:], v["wq"][dt_i * 128:(dt_i + 1) * 128, :])
            wq_t.append(wt)
            xt_ = xin_pool.tile([128, 1024], BF16, tag="xin",
                                name=f"qpx{dt_i}")
            eng = nc.sync if dt_i % 2 == 0 else nc.scalar
            eng.dma_start(xt_[:], v["xqp"][dt_i * 128:(dt_i + 1) * 128, :])
            qp_x.append(xt_)
            wt = w_pool.tile([128, CH], BF16, tag="wk", bufs=NDT,
                             name=f"wk{dt_i}")
            nc.scalar.dma_start(wt[:], v["wk"][dt_i * 128:(dt_i + 1) * 128, :])
            wk_t.append(wt)
        for name, src, w_t, dst in (("qp", "xqp", wq_t, qp_sb),
                                    ("qm", "xqm", wq_t, qm_sb),
                                    ("kp", "xkp", wk_t, kp_sb),
                                    ("km", "xkm", wk_t, km_sb)):
            if name == "qp":
                x_t = qp_x
            else:
                x_t = []
                for dt_i in range(NDT):
                    xt_ = xin_pool.tile([128, 1024], BF16, tag="xin",
                                        name=f"{name}x{dt_i}")
                    eng = nc.sync if dt_i % 2 == 0 else nc.scalar
                    eng.dma_start(xt_[:],
                                  v[src][dt_i * 128:(dt_i + 1) * 128, :])
                    x_t.append(xt_)
            for ltg in range(2):
                ps = [psum_a.tile([128, CH], F32, tag="ps", bufs=6,
                                  name=f"p{name}{ltg}_{i}") for i in range(4)]
                for dt_i in range(NDT):
                    for li in range(4):
                        lt = ltg * 4 + li
                        nc.tensor.matmul(
                            ps[li][:],
                            x_t[dt_i][:, lt * 128:(lt + 1) * 128],
                            w_t[dt_i][:], start=(dt_i == 0),
                            stop=(dt_i == NDT - 1))
                for li in range(4):
                    t = qk_pool.tile([128, CH], BF16, tag="qk",
                                     name=f"{name}{ltg * 4 + li}")
                    nc.scalar.copy(t[:], ps[li][:])
                    dst[ltg * 4 + li] = t

        # q0/k0 rows (raw l=0 projections, no bias) + broadcast tiles
        ps_q0 = psum_a.tile([1, CH], F32, tag="ps0", bufs=2)
        ps_k0 = psum_a.tile([1, CH], F32, tag="ps0", bufs=2)
        for dt_i in range(NDT):
            nc.tensor.matmul(ps_q0[:], xq0_sb[:, dt_i:dt_i + 1], wq_t[dt_i][:],
                             start=(dt_i == 0), stop=(dt_i == NDT - 1))
        for dt_i in range(NDT):
            nc.tensor.matmul(ps_k0[:], xk0_sb[:, dt_i:dt_i + 1], wk_t[dt_i][:],
                             start=(dt_i == 0), stop=(dt_i == NDT - 1))
        nc.scalar.copy(q0row[:], ps_q0[:])
        nc.scalar.copy(k0row[:], ps_k0[:])
        ps_qb = psum_a.tile([128, CH], F32, tag="ps", bufs=6, name="psqb")
        ps_kb = psum_a.tile([128, CH], F32, tag="ps", bufs=6, name="pskb")
        nc.tensor.matmul(ps_qb[:], ones_sb[:], q0row[:], start=True, stop=True)
        nc.tensor.matmul(ps_kb[:], ones_sb[:], k0row[:], start=True, stop=True)
        nc.scalar.copy(q0bc[:], ps_qb[:])
        nc.scalar.copy(k0bc[:], ps_kb[:])
        # chunk-0 correction tile: q0bc + L*bq on the f=0 partition row
        nc.vector.tensor_copy(q0bc0[:], q0bc[:])
        nc.vector.tensor_add(q0bc0[0:1, :], q0bc[0:1, :], bqL_sb[:])

    # =============== Stage B: folded DFT + channel-summed products =========
    # trig chunks 0/1 prefetch first, then the value-path weights/inputs so
    # they sit ahead of the slot-blocked later trig chunks in the DMA queues
    trig_pool = stack.enter_context(tc.tile_pool(name="trigB", bufs=6))
    trig_tiles = {}
    for ft in range(2):
        cos_sb = trig_pool.tile([128, NLT * 128], BF16, tag="trig",
                                name=f"cos{ft}")
        nc.sync.dma_start(cos_sb[:], v["cosF8"][ft])
        sin_sb = trig_pool.tile([128, NLT * 128], BF16, tag="trig",
                                name=f"sin{ft}")
        nc.sync.dma_start(sin_sb[:], v["sinF8"][ft])
        trig_tiles[ft] = (cos_sb, sin_sb)
    wf_pool = stack.enter_context(tc.tile_pool(name="wf", bufs=1))
    xtv_pool = stack.enter_context(tc.tile_pool(name="xtv", bufs=1))
    wvt_t, wo_t, xtv_t, wf_t = [], [], [], []
    for ct in range(NDT):
        t = wf_pool.tile([128, D], BF16, tag="wvt", bufs=NDT, name=f"wvt{ct}")
        nc.sync.dma_start(t[:], v["wvt"][ct * 128:(ct + 1) * 128, :])
        wvt_t.append(t)
        t = wf_pool.tile([128, CH], BF16, tag="wo", bufs=NDT, name=f"wo{ct}")
        nc.scalar.dma_start(t[:], v["wo"][ct * 128:(ct + 1) * 128, :])
        wo_t.append(t)
        t = xtv_pool.tile([128, L], BF16, tag="xtv", bufs=NDT, name=f"xtv{ct}")
        eng = nc.sync if ct % 2 == 0 else nc.scalar
        eng.dma_start(t[:], v["xtv"][ct * 128:(ct + 1) * 128, :])
        xtv_t.append(t)

    with tc.tile_pool(name="ev", bufs=1) as ev_pool, \
         tc.tile_pool(name="psumB", bufs=8, space="PSUM") as psum_b:
        for ft in range(NFT):
            if ft < 2:
                cos_sb, sin_sb = trig_tiles[ft]
            else:
                cos_sb = trig_pool.tile([128, NLT * 128], BF16, tag="trig",
                                        name=f"cos{ft}")
                nc.sync.dma_start(cos_sb[:], v["cosF8"][ft])
                sin_sb = trig_pool.tile([128, NLT * 128], BF16, tag="trig",
                                        name=f"sin{ft}")
                nc.sync.dma_start(sin_sb[:], v["sinF8"][ft])

            pA = psum_b.tile([128, CH], F32, tag="ps", bufs=6, name="pA")
            pC = psum_b.tile([128, CH], F32, tag="ps", bufs=6, name="pC")
            pAs = psum_b.tile([128, CH], F32, tag="ps", bufs=6, name="pAs")
            pCs = psum_b.tile([128, CH], F32, tag="ps", bufs=6, name="pCs")
            for lt in range(NLT):
                st, sp = (lt == 0), (lt == NLT - 1)
                cs = cos_sb[:, lt * 128:(lt + 1) * 128]
                ss = sin_sb[:, lt * 128:(lt + 1) * 128]
                nc.tensor.matmul(pA[:], cs, qp_sb[lt][:], start=st, stop=sp)
                nc.tensor.matmul(pC[:], cs, kp_sb[lt][:], start=st, stop=sp)
                nc.tensor.matmul(pAs[:], ss, qm_sb[lt][:], start=st, stop=sp)
                nc.tensor.matmul(pCs[:], ss, km_sb[lt][:], start=st, stop=sp)

            eA = ev_pool.tile([128, CH], F32, tag="ev", bufs=6)
            eC = ev_pool.tile([128, CH], F32, tag="ev", bufs=6)
            eAs = ev_pool.tile([128, CH], F32, tag="ev", bufs=6)
            eCs = ev_pool.tile([128, CH], F32, tag="ev", bufs=6)
            nc.scalar.copy(eA[:], pA[:])
            nc.scalar.copy(eC[:], pC[:])
            nc.scalar.copy(eAs[:], pAs[:])
            nc.scalar.copy(eCs[:], pCs[:])
            eA2 = ev_pool.tile([128, CH], F32, tag="ev2", bufs=4)
            eC2 = ev_pool.tile([128, CH], F32, tag="ev2", bufs=4)
            nc.vector.tensor_add(eA2[:], eA[:], (q0bc0 if ft == 0 else q0bc)[:])
            nc.vector.tensor_add(eC2[:], eC[:], k0bc[:])
            prodR = ev_pool.tile([128, CH], F32, tag="prod", bufs=2)
            prodS = ev_pool.tile([128, CH], F32, tag="prod", bufs=2)
            nc.vector.tensor_mul(prodR[:], eA2[:], eC2[:])
            nc.vector.tensor_mul(prodS[:], eAs[:], eCs[:])
            nc.vector.reduce_sum(rs_sb[:, ft:ft + 1], prodR[:],
                                 axis=mybir.AxisListType.X)
            nc.vector.reduce_sum(rs_sb[:, NFT + ft:NFT + ft + 1], prodS[:],
                                 axis=mybir.AxisListType.X)

        # Nyquist bin f=1024 (exact): A/C via alternating-sign column
        pN1 = psum_b.tile([1, CH], F32, tag="psn", bufs=2, name="pN1")
        pN2 = psum_b.tile([1, CH], F32, tag="psn", bufs=2, name="pN2")
        for lt in range(NLT):
            st, sp = (lt == 0), (lt == NLT - 1)
            nc.tensor.matmul(pN1[:], altcol_sb[:, lt:lt + 1], qp_sb[lt][:],
                             start=st, stop=sp)
        for lt in range(NLT):
            st, sp = (lt == 0), (lt == NLT - 1)
            nc.tensor.matmul(pN2[:], altcol_sb[:, lt:lt + 1], kp_sb[lt][:],
                             start=st, stop=sp)
        eN1 = ev_pool.tile([1, CH], F32, tag="en", bufs=4)
        eN2 = ev_pool.tile([1, CH], F32, tag="en", bufs=4)
        nc.scalar.copy(eN1[:], pN1[:])
        nc.scalar.copy(eN2[:], pN2[:])
        eN1b = ev_pool.tile([1, CH], F32, tag="en", bufs=4)
        eN2b = ev_pool.tile([1, CH], F32, tag="en", bufs=4)
        nc.vector.tensor_add(eN1b[:], eN1[:], q0row[:])
        nc.vector.tensor_add(eN2b[:], eN2[:], k0row[:])
        prodN = ev_pool.tile([1, CH], F32, tag="pn")
        nc.vector.tensor_mul(prodN[:], eN1b[:], eN2b[:])
        nc.vector.memset(rs_sb[:, 2 * NFT:2 * NFT + 1], 0.0)
        nc.vector.reduce_sum(rs_sb[0:1, 2 * NFT:2 * NFT + 1], prodN[:],
                             axis=mybir.AxisListType.X)

    # =============== Stage C: pairwise all-reduce of (R,S) ===============
    nc.gpsimd.dma_start(v["rs_in"][:], rs_sb[:])
    nc.gpsimd.collective_compute(
        "AllReduce", mybir.AluOpType.add,
        replica_groups=[[0, 1], [2, 3], [4, 5], [6, 7]],
        ins=[v["rs_in"].opt()], outs=[v["rs_out"].opt()])
    nc.gpsimd.dma_start(rs2_sb[:], v["rs_out"][:])

    psum_def = stack.enter_context(
        tc.tile_pool(name="psumDEF", bufs=8, space="PSUM"))
    vpt_pool = stack.enter_context(tc.tile_pool(name="vpt", bufs=1))
    vpt2 = [vpt_pool.tile([128, 2 * L], BF16, tag="vpt2", bufs=4,
                          name=f"vpt2_{i}") for i in range(4)]

    # =============== Stage W: fused Wv@Wo + value transform ===============
    # (issued after the collective fire so the PE chews it during the
    #  allreduce + reload window)
    for dt_i in range(NDT):
        ps = psum_def.tile([128, CH], F32, tag="ps", bufs=6, name=f"pwf{dt_i}")
        for ct in range(NDT):
            nc.tensor.matmul(ps[:], wvt_t[ct][:, dt_i * 128:(dt_i + 1) * 128],
                             wo_t[ct][:], start=(ct == 0), stop=(ct == NDT - 1))
        t = wf_pool.tile([128, CH], BF16, tag="wf", bufs=NDT, name=f"wf{dt_i}")
        nc.scalar.copy(t[:], ps[:])
        wf_t.append(t)
    # delta row = bv @ Wo_half -> [1,512] -> DRAM -> [128,4] -> bo2
    ps_d = psum_def.tile([1, CH], F32, tag="small", bufs=2, name="ps_d")
    for ct in range(NDT):
        nc.tensor.matmul(ps_d[:], bv_sb[:, ct:ct + 1], wo_t[ct][:],
                         start=(ct == 0), stop=(ct == NDT - 1))
    drow = const_pool.tile([1, CH], F32, tag="drow")
    nc.scalar.copy(drow[:], ps_d[:])
    nc.scalar.dma_start(v["delta_dram"][:], drow[:])
    dcols = const_pool.tile([128, 4], F32, tag="dcols")
    nc.scalar.dma_start(
        dcols[:], v["delta_dram"].rearrange("o (a p) -> p (o a)", p=128))
    nc.vector.tensor_add(bo2_sb[:], bo_sb[:], dcols[:])

    def vpt_jtile(jt, on_vector=False):
        for lch in range(4):
            ps = psum_def.tile([128, 512], F32, tag="ps", bufs=6, name=f"pv{jt}_{lch}")
            for dt_i in range(NDT):
                nc.tensor.matmul(
                    ps[:], wf_t[dt_i][:, jt * 128:(jt + 1) * 128],
                    xtv_t[dt_i][:, lch * 512:(lch + 1) * 512],
                    start=(dt_i == 0), stop=(dt_i == NDT - 1))
            if on_vector:
                nc.vector.tensor_copy(vpt2[jt][:, lch * 512:(lch + 1) * 512],
                                      ps[:])
            else:
                nc.scalar.copy(vpt2[jt][:, lch * 512:(lch + 1) * 512], ps[:])
            eng = nc.sync if lch % 2 == 0 else nc.scalar
            eng.dma_start(vpt2[jt][:, L + lch * 512:L + (lch + 1) * 512],
                          vpt2[jt][:, lch * 512:(lch + 1) * 512])

    vpt_jtile(0)
    vpt_jtile(1)

    # =============== Stage D: irfft (factored) + top-16 + weights ==========
    with tc.tile_pool(name="trigD", bufs=4) as trigd_pool, \
         tc.tile_pool(name="top", bufs=1) as top_pool:
        rrep = top_pool.tile([128, 4 * NFT], F32, tag="rrep")
        srep = top_pool.tile([128, 4 * NFT], F32, tag="srep")
        for ft in range(NFT):
            nc.vector.tensor_copy(
                rrep[:, ft * 4:(ft + 1) * 4],
                rs2_sb[:, ft:ft + 1].to_broadcast((128, 4)))
            nc.vector.tensor_copy(
                srep[:, ft * 4:(ft + 1) * 4],
                rs2_sb[:, NFT + ft:NFT + ft + 1].to_broadcast((128, 4)))
        t1 = top_pool.tile([128, 4 * NFT], F32, tag="t1")
        t2 = top_pool.tile([128, 4 * NFT], F32, tag="t2")
        uu = top_pool.tile([128, 4 * NFT], F32R, tag="uu")
        vv = top_pool.tile([128, 4 * NFT], F32R, tag="vv")
        nc.vector.tensor_mul(t1[:], rrep[:], wca_sb[:])
        nc.vector.tensor_mul(t2[:], srep[:], wsa_sb[:])
        nc.vector.tensor_add(uu[:], t1[:], t2[:])
        nc.vector.tensor_mul(t1[:], srep[:], wca_sb[:])
        nc.vector.tensor_mul(t2[:], rrep[:], wsa_sb[:])
        nc.vector.tensor_sub(vv[:], t1[:], t2[:])
        nyqrow = top_pool.tile([1, 4], F32R, tag="nyq")
        nc.vector.tensor_copy(
            nyqrow[:], rs2_sb[0:1, 2 * NFT:2 * NFT + 1].to_broadcast((1, 4)))

        mv_ps = psum_def.tile([4, 512], F32, tag="small", bufs=2, name="mvps")
        for ft in range(NFT):
            cb_sb = trigd_pool.tile([128, 512], F32R, tag="trig")
            nc.sync.dma_start(cb_sb[:], v["cosB3"][ft])
            sb_sb = trigd_pool.tile([128, 512], F32R, tag="trig")
            nc.sync.dma_start(sb_sb[:], v["sinB3"][ft])
            nc.tensor.matmul(mv_ps[:], uu[:, ft * 4:(ft + 1) * 4], cb_sb[:],
                             start=(ft == 0), stop=False)
            nc.tensor.matmul(mv_ps[:], vv[:, ft * 4:(ft + 1) * 4], sb_sb[:],
                             start=False, stop=False)
        nc.tensor.matmul(mv_ps[:], nyqrow[:], altb_sb[:],
                         start=False, stop=True)

        mv4 = top_pool.tile([4, 512], F32, tag="mv4")
        nc.vector.tensor_copy(mv4[:], mv_ps[:])
        mv_sb = top_pool.tile([1, L], F32, tag="mv")
        nc.gpsimd.dma_start(
            v["mv_dram"].rearrange("o (a b) -> a (o b)", a=4), mv4[:])
        nc.gpsimd.dma_start(mv_sb[:], v["mv_dram"][:])

        # top-k round 1: top-8 values; the gather's first tap wave can
        # start on unnormalized exp weights (the 1/sum scale is applied at
        # the output activation), overlapping round 2 with PE work.  The
        # weight chain (sub/exp/esr) is issued ahead of FIND_INDEX8 and of
        # the VPT evacuations so no queue blocks it.
        vals16 = top_pool.tile([1, 16], F32, tag="vals")
        idx16 = top_pool.tile([1, 16], U32, tag="idx")
        mv_m = top_pool.tile([1, L], F32, tag="mvm")
        m1 = vals16[0:1, 0:8]
        m2 = vals16[0:1, 8:16]
        es = top_pool.tile([1, 18], F32, tag="es")
        esr = top_pool.tile([1, 18], F32R, tag="esr")
        wbs = top_pool.tile([128, 18], F32, tag="wbs")

        nc.vector.max(m1, mv_sb[:])
        nc.vector.tensor_sub(es[0:1, 0:8], m1,
                             vals16[0:1, 0:1].to_broadcast((1, 8)))
        nc.scalar.activation(es[0:1, 0:8], es[0:1, 0:8], AF.Exp)
        nc.vector.tensor_copy(esr[0:1, 0:8], es[0:1, 0:8])
        nc.vector.max_index(idx16[0:1, 0:8], m1, mv_sb[:])

        vpt_jtile(2)
        vpt_jtile(3, on_vector=True)

        wb1 = psum_def.tile([128, 8], F32, tag="small", bufs=2, name="wb1")
        nc.tensor.matmul(wb1[:], ones_sb[:], esr[0:1, 0:8],
                         start=True, stop=True)
        nc.scalar.copy(wbs[:, 0:8], wb1[:])
        for j in range(8):
            nc.scalar.mul(wI[:, j * 128:(j + 1) * 128], ident_sb[:],
                          wbs[:, j:j + 1])
        _, deltas1 = nc.values_load_multi_w_load_instructions(
            idx16[0:1, 0:8], engines=(mybir.EngineType.PE,),
            min_val=0, max_val=L - 1, skip_runtime_bounds_check=True)

        # top-k round 2 (runs on DVE while the PE does tap wave 1)
        nc.vector.match_replace(mv_m[:], m1, mv_sb[:], -1e30)
        nc.vector.max(m2, mv_m[:])
        nc.vector.max_index(idx16[0:1, 8:16], m2, mv_m[:])
        nc.vector.tensor_sub(es[0:1, 8:16], m2,
                             vals16[0:1, 0:1].to_broadcast((1, 8)))
        nc.scalar.activation(es[0:1, 8:16], es[0:1, 8:16], AF.Exp)
        nc.vector.memset(es[0:1, 15:16], 0.0)
        nc.vector.reduce_sum(es[0:1, 16:17], es[0:1, 0:16],
                             axis=mybir.AxisListType.X)
        nc.vector.memset(es[0:1, 17:18], 0.0)
        nc.vector.tensor_copy(esr[0:1, 8:18], es[0:1, 8:18])

    # =============== Stage F: gather (15 taps, two waves) + output =========
    grp_tiles = [(jt, nch) for jt in range(4) for nch in range(4)]
    groups = [grp_tiles[0:6], grp_tiles[6:12], grp_tiles[12:16]]
    with tc.tile_pool(name="outp", bufs=1) as out_pool:
        deltas2 = None
        for gi, grp in enumerate(groups):
            pss = []
            for (jt, nch) in grp:
                ps = psum_def.tile([128, 512], F32, tag="ps", bufs=6,
                                   name=f"pg{jt}_{nch}")
                pss.append(ps)
            for j in range(8):
                for ps, (jt, nch) in zip(pss, grp):
                    nc.tensor.matmul(
                        ps[:], wI[:, j * 128:(j + 1) * 128],
                        vpt2[jt][:, bass.ds(deltas1[j] + nch * 512, 512)],
                        start=(j == 0), stop=False)
            if gi == 0:
                # round-2 weights: broadcast + wI build + register loads
                wb2 = psum_def.tile([128, 10], F32, tag="small", bufs=2,
                                    name="wb2")
                nc.tensor.matmul(wb2[:], ones_sb[:], esr[0:1, 8:18],
                                 start=True, stop=True)
                nc.vector.tensor_copy(wbs[:, 8:18], wb2[:])
                nc.vector.reciprocal(inv_sb[:], wbs[:, 16:17])
                for j in range(8, NTAP):
                    nc.vector.tensor_scalar_mul(
                        wI[:, j * 128:(j + 1) * 128], ident_sb[:],
                        wbs[:, j:j + 1])
                _, deltas2 = nc.values_load_multi_w_load_instructions(
                    idx16[0:1, 8:NTAP], engines=(mybir.EngineType.PE,),
                    min_val=0, max_val=L - 1,
                    skip_runtime_bounds_check=True)
            for j in range(8, NTAP):
                for ps, (jt, nch) in zip(pss, grp):
                    nc.tensor.matmul(
                        ps[:], wI[:, j * 128:(j + 1) * 128],
                        vpt2[jt][:, bass.ds(deltas2[j - 8] + nch * 512, 512)],
                        start=False, stop=(j == NTAP - 1))
            for ps, (jt, nch) in zip(pss, grp):
                o = out_pool.tile([128, 512], BF16, tag="oev", bufs=6)
                nc.scalar.activation(o[:], ps[:], AF.Identity,
                                     bias=bo2_sb[:, jt:jt + 1],
                                     scale=inv_sb[:])
                eng = nc.sync if (jt * 4 + nch) % 2 == 0 else nc.scalar
                eng.dma_start(
                    v["out_t"][jt * 128:(jt + 1) * 128,
                               nch * 512:(nch + 1) * 512], o[:])

    stack.close()


def _get_program():
    if "nc" not in _cache:
        _cache["nc"] = _build_program()
    return _cache["nc"]


def _fold(x):
    """x: [D, L] fp32 -> (x+, x-, x0col) folded per DFT even/odd symmetry."""
    xp = np.empty((D, 1024), np.float32)
    xm = np.empty((D, 1024), np.float32)
    xp[:, :1023] = x[:, 1:1024] + x[:, 2047:1024:-1]
    xm[:, :1023] = x[:, 1:1024] - x[:, 2047:1024:-1]
    xp[:, 1023] = x[:, 1024]
    xm[:, 1023] = 0.0
    x0 = np.ascontiguousarray(x[:, 0].reshape(NDT, 128).T)   # [128, 8]
    return xp.astype(NPBF16), xm.astype(NPBF16), x0.astype(NPBF16)


def kernel(queries, keys, values, Wq, bq, Wk, bk, Wv, bv, Wo, bo):
    queries = np.asarray(queries, np.float32)
    keys = np.asarray(keys, np.float32)
    values = np.asarray(values, np.float32)
    Wq = np.asarray(Wq, np.float32); bq = np.asarray(bq, np.float32)
    Wk = np.asarray(Wk, np.float32); bk = np.asarray(bk, np.float32)
    Wv = np.asarray(Wv, np.float32); bv = np.asarray(bv, np.float32)
    Wo = np.asarray(Wo, np.float32); bo = np.asarray(bo, np.float32)

    (cosF8, sinF8, altcol, wcaP, wsaP, cosB3, sinB3,
     altb_row) = _cache.setdefault("const", _host_constants())
    ones_row = np.ones((1, 128), np.float32)
    ident = np.eye(128, dtype=np.float32).astype(NPBF16)
    wvt = np.ascontiguousarray(Wv.T).astype(NPBF16)
    bv_dt = np.ascontiguousarray(bv.reshape(NDT, 128).T).astype(NPBF16)

    per_batch = []
    for b in range(B):
        xq = np.ascontiguousarray(queries[b].T)
        xk = np.ascontiguousarray(keys[b].T)
        xtv = np.ascontiguousarray(values[b].T).astype(NPBF16)
        per_batch.append((_fold(xq), _fold(xk), xtv))

    in_maps = []
    for core in range(N_CORES):
        b, half = core // 2, core % 2
        cs = slice(half * CH, (half + 1) * CH)
        (xqp, xqm, xq0), (xkp, xkm, xk0), xtv = per_batch[b]
        in_maps.append({
            "xqp": xqp, "xqm": xqm, "xq0": xq0,
            "xkp": xkp, "xkm": xkm, "xk0": xk0,
            "xtv": xtv,
            "wq": np.ascontiguousarray(Wq[:, cs]).astype(NPBF16),
            "wk": np.ascontiguousarray(Wk[:, cs]).astype(NPBF16),
            "wvt": wvt,
            "wo": np.ascontiguousarray(Wo[:, cs]).astype(NPBF16),
            "bv_dt": bv_dt,
            "bqL_row": np.ascontiguousarray((L * bq[cs])[None, :]),
            "bkL_row": np.ascontiguousarray((L * bk[cs])[None, :]),
            "bo_cols": np.ascontiguousarray(bo[cs].reshape(4, 128).T),
            "cosF8": cosF8, "sinF8": sinF8, "altcol": altcol,
            "cosB3": cosB3, "sinB3": sinB3,
            "wcaP": wcaP, "wsaP": wsaP, "altb_row": altb_row,
            "ones_row": ones_row, "ident": ident,
        })

    nc = _get_program()
    res = run_bass_kernel_spmd(nc, in_maps, core_ids=list(range(N_CORES)),
                               **_cache.get("run_kwargs", {}))
    _cache["last_result"] = res

    out = np.empty((B, L, D), np.float32)
    for core in range(N_CORES):
        b, half = core // 2, core % 2
        out[b, :, half * CH:(half + 1) * CH] = \
            res.results[core]["out_t"].T.astype(np.float32)
    return out
